# revision 1
# baseline (speedup 1.0000x reference)
"""Trainium2 Bass kernel for the BinaryMechanismSSM problem.

Full inputs in, full outputs out. Internally: batch (128) sharded 8 ways
(16 rows/core). Per core:
  Phase 1: projections bx0/bx1/gx = x @ {B0,B1,G}^T + bias (f32r matmuls,
           N=512 token tiles), sigmoid applied to the gate, staged to DRAM
           planes proj[mat][j] = [128, T*16] (token = t*16 + b).
  Phase 2: T sequential steps. State held as st[p, 16j+b] = s[b, 128j+p]
           ([128, 64] tile). Per step: 32 fp16 matmuls (weight-stationary
           A-blocks, rhs = fp16 state slices) accumulate f0/f1 into
           [128, 64] PSUM; DVE adds the staged projections; ACT tanh;
           DVE blend + gate; per-step DMA of the new state to a staging
           buffer [T, 128, 64]. Host re-layouts to [B, T+1, S].
"""
import numpy as np

B_FULL = 128
T_FULL = 1024
I_DIM = 256
S_DIM = 512
N_CORES = 8
B_LOC = B_FULL // N_CORES  # 16

_cache = {}


def _build(alpha: float, z: int, T: int):
    import concourse.bass as bass
    from concourse import bacc
    import concourse.mybir as mybir
    from concourse.tile import TileContext

    dt = mybir.dt
    AF = mybir.ActivationFunctionType
    ALU = mybir.AluOpType

    TOK = T * B_LOC          # tokens per core
    NTT = TOK // 512         # phase-1 token tiles
    NG = T // 16             # phase-2 step groups
    NMAT = 3 if z != 0 else 2          # number of projection matrices
    NREC = 2 if z != 0 else 1          # number of recurrence matrices

    nc = bacc.Bacc("TRN2", target_bir_lowering=False, debug=False,
                   num_devices=N_CORES)

    xT_d = nc.declare_dram_parameter("xT", [2, 128, TOK], dt.float32r, isOutput=False)
    pw_d = nc.declare_dram_parameter("pw", [128, NMAT * 2 * 4 * 128], dt.float32r, isOutput=False)
    bias_d = nc.declare_dram_parameter("bias", [128, 4 * NMAT], dt.float32, isOutput=False)
    aw_d = nc.declare_dram_parameter("aw", [128, NREC * 16 * 128], dt.float16, isOutput=False)
    s0_d = nc.declare_dram_parameter("s0T", [128, 64], dt.float32, isOutput=False)
    iden_d = nc.declare_dram_parameter("iden", [128, 128], dt.float16, isOutput=False)
    stg_d = nc.declare_dram_parameter("stg", [T, 128, 64], dt.float32, isOutput=True)

    with TileContext(nc) as tc:
      with tc.tile_pool(name="dram", bufs=1, space="DRAM") as dpool:
        projh_p = [[dpool.tile([128, TOK], dt.float16, tag=f"projh{m}{j}",
                               name=f"projh{m}{j}")
                    for j in range(4)] for m in range(NREC)]
        projl_p = [[dpool.tile([128, TOK], dt.float16, tag=f"projl{m}{j}",
                               name=f"projl{m}{j}")
                    for j in range(4)] for m in range(NREC)]
        projg_p = [dpool.tile([128, TOK], dt.float32, tag=f"projg{j}",
                              name=f"projg{j}") for j in range(4)]
        # ---------------- Phase 1: projections ----------------
        with (
            tc.tile_pool(name="p1w", bufs=1) as p1w,
            tc.tile_pool(name="p1x", bufs=3) as p1x,
            tc.tile_pool(name="p1o", bufs=6) as p1o,
            tc.tile_pool(name="p1ps", bufs=8, space="PSUM") as p1ps,
        ):
            pw = p1w.tile([128, NMAT * 2 * 4 * 128], dt.float32r)
            nc.sync.dma_start(pw[:], pw_d[:])
            bias = p1w.tile([128, 4 * NMAT], dt.float32)
            nc.sync.dma_start(bias[:], bias_d[:])

            for tt in range(NTT):
                xt = p1x.tile([128, 2 * 512], dt.float32r, tag="xt")
                for i in range(2):
                    nc.sync.dma_start(xt[:, i * 512:(i + 1) * 512],
                                      xT_d[i, :, tt * 512:(tt + 1) * 512])
                for mat in range(NMAT):
                    for j in range(4):
                        ps = p1ps.tile([128, 512], dt.float32, tag="pps")
                        for i in range(2):
                            blk = ((mat * 2 + i) * 4 + j) * 128
                            nc.tensor.matmul(
                                ps[:], pw[:, blk:blk + 128],
                                xt[:, i * 512:(i + 1) * 512],
                                start=(i == 0), stop=(i == 1))
                        bj = bias[:, mat * 4 + j:mat * 4 + j + 1]
                        if mat == NMAT - 1:
                            ot = p1o.tile([128, 512], dt.float32, tag="po")
                            nc.scalar.activation(ot[:], ps[:], AF.Sigmoid,
                                                 bias=bj, scale=1.0)
                            nc.sync.dma_start(
                                projg_p[j][:, tt * 512:(tt + 1) * 512], ot[:])
                        else:
                            hi = p1o.tile([128, 512], dt.float16, tag="phi")
                            nc.scalar.activation(hi[:], ps[:], AF.Identity,
                                                 bias=bj, scale=1.0)
                            lo = p1o.tile([128, 512], dt.float16, tag="plo")
                            nc.vector.scalar_tensor_tensor(
                                lo[:], ps[:], bj, hi[:], ALU.add, ALU.subtract)
                            nc.sync.dma_start(
                                projh_p[mat][j][:, tt * 512:(tt + 1) * 512], hi[:])
                            nc.sync.dma_start(
                                projl_p[mat][j][:, tt * 512:(tt + 1) * 512], lo[:])

        # ---------------- Phase 2: recurrence ----------------
        with (
            tc.tile_pool(name="p2w", bufs=1) as p2w,
            tc.tile_pool(name="p2in", bufs=2) as p2in,
            tc.tile_pool(name="p2st", bufs=2) as p2st,
            tc.tile_pool(name="p2c", bufs=3) as p2c,
            tc.tile_pool(name="p2ps", bufs=4, space="PSUM") as p2ps,
        ):
            aw = p2w.tile([128, NREC * 16 * 128], dt.float16)
            nc.sync.dma_start(aw[:], aw_d[:])
            iden = p2w.tile([128, 128], dt.float16)
            nc.sync.dma_start(iden[:], iden_d[:])

            st = p2st.tile([128, 64], dt.float32, tag="st")
            nc.sync.dma_start(st[:], s0_d[:])
            st16 = p2st.tile([128, 64], dt.float16, tag="st16")
            nc.scalar.activation(st16[:], st[:], AF.Copy)

            GATE_MAT = NMAT - 1
            a0 = float(1.0 - alpha) if z != 0 else 1.0
            a1 = float(alpha)

            for g in range(NG):
                # staged bx planes (f32r for the identity-MM injection)
                # contiguous hi/lo fp16 staging: (h, m, j, t, b)
                pjb = p2in.tile([128, 2 * NREC * 4 * 256], dt.float16, tag="pjb")
                for h, planes in enumerate((projh_p, projl_p)):
                    for m in range(NREC):
                        for j in range(4):
                            nc.sync.dma_start(
                                pjb[:, ((h * NREC + m) * 4 + j) * 256:
                                       ((h * NREC + m) * 4 + j + 1) * 256],
                                planes[m][j][:, g * 256:(g + 1) * 256])
                pjbr = pjb[:].rearrange("p (h m j t b) -> p h m j t b",
                                        h=2, m=NREC, j=4, t=16, b=16)
                # staged gate plane (fp32 for DVE)
                pjg = p2in.tile([128, 4 * 256], dt.float32, tag="pjg")
                for j in range(4):
                    nc.sync.dma_start(
                        pjg[:, j * 256:(j + 1) * 256],
                        projg_p[j][:, g * 256:(g + 1) * 256])

                # per-group gate coefficient planes (off the serial path):
                # gco[:, m-block] = coef_m * g ; g1m = 1 - g
                gco = p2in.tile([128, NREC * 1024], dt.float32, tag="gco")
                nc.vector.tensor_scalar_mul(gco[:, 0:1024], pjg[:], a0)
                if NREC == 2:
                    nc.vector.tensor_scalar_mul(gco[:, 1024:2048], pjg[:], a1)
                gcor = gco[:].rearrange("p (m j t b) -> p m j t b",
                                        m=NREC, j=4, t=16, b=16)
                g1m = p2in.tile([128, 1024], dt.float32, tag="g1m")
                nc.vector.tensor_scalar(g1m[:], pjg[:], -1.0, 1.0,
                                        ALU.mult, ALU.add)
                g1mr = g1m[:].rearrange("p (j t b) -> p j t b", j=4, t=16)

                for tt in range(16):
                    t = g * 16 + tt
                    W = NREC * 64
                    pscat = p2ps.tile([128, W], dt.float32, tag="pscat")
                    # inject bx = hi + lo via fp16 identity matmuls
                    for m in range(NREC):
                        for h in range(2):
                            nc.tensor.matmul(
                                pscat[:, m * 64:(m + 1) * 64]
                                .rearrange("p (j b) -> p j b", j=4),
                                iden[:], pjbr[:, h, m, :, tt, :],
                                start=(m == 0 and h == 0), stop=False)
                    # m2 = (1-g) * s  (off serial path, only needs st)
                    m2 = p2c.tile([128, 64], dt.float32, tag="m2")
                    nc.vector.tensor_tensor(
                        m2[:].rearrange("p (j b) -> p j b", j=4),
                        st[:].rearrange("p (j b) -> p j b", j=4),
                        g1mr[:, :, tt, :], ALU.mult)
                    # A matmuls accumulate on top
                    for m in range(NREC):
                        for j in range(4):
                            for k in range(4):
                                blk = (m * 16 + k * 4 + j) * 128
                                nc.tensor.matmul(
                                    pscat[:, (m * 4 + j) * 16:(m * 4 + j + 1) * 16],
                                    aw[:, blk:blk + 128],
                                    st16[:, k * 16:(k + 1) * 16],
                                    start=False,
                                    stop=(k == 3))
                    # one tanh over the whole [128, NREC*64] psum
                    ft = p2c.tile([128, W], dt.float32, tag="ft")
                    nc.scalar.activation(ft[:], pscat[:], AF.Tanh)
                    # mcat = gco_t * ft
                    mc = p2c.tile([128, W], dt.float32, tag="mc")
                    nc.vector.tensor_tensor(
                        mc[:].rearrange("p (m j b) -> p m j b", m=NREC, j=4),
                        ft[:].rearrange("p (m j b) -> p m j b", m=NREC, j=4),
                        gcor[:, :, :, tt, :], ALU.mult)
                    # reduce + new state (fp16 copy gates next step's matmuls)
                    if NREC == 2:
                        t2 = p2c.tile([128, 64], dt.float32, tag="t2")
                        nc.vector.tensor_tensor(t2[:], mc[:, 0:64], mc[:, 64:128],
                                                ALU.add)
                    else:
                        t2 = mc
                    st16_new = p2st.tile([128, 64], dt.float16, tag="st16")
                    nc.vector.tensor_tensor(st16_new[:], t2[:], m2[:], ALU.add)
                    st_new = p2st.tile([128, 64], dt.float32, tag="st")
                    nc.vector.tensor_tensor(st_new[:], t2[:], m2[:], ALU.add)
                    st, st16 = st_new, st16_new

                    nc.sync.dma_start(stg_d[t], st[:])

    nc.compile()
    return nc


def _pack_lhsT_blocks(W, kdim, mdim, dtype):
    """W: [mdim*128, kdim*128]; returns [128, kdim*mdim*128] with block
    (k, j) at cols (k*mdim+j)*128 equal to W[j-chunk, k-chunk].T."""
    nk, nj = kdim, mdim
    out = np.zeros((128, nk * nj * 128), dtype=dtype)
    for k in range(nk):
        for j in range(nj):
            blk = W[j * 128:(j + 1) * 128, k * 128:(k + 1) * 128].T
            out[:, (k * nj + j) * 128:(k * nj + j + 1) * 128] = blk
    return np.ascontiguousarray(out)


def kernel(x_seq, s0, A0_w, B0_w, B0_b, A1_w, B1_w, B1_b, gate_w, gate_b,
           alpha, z, _T=None, _trace=False):
    from concourse.bass_utils import run_bass_kernel_spmd

    T = int(_T or T_FULL)
    alpha_f = float(np.asarray(alpha))
    z_i = int(np.asarray(z))

    key = (alpha_f, z_i, T)
    if key not in _cache:
        _cache[key] = _build(alpha_f, z_i, T)
    nc = _cache[key]

    NMAT = 3 if z_i != 0 else 2
    NREC = 2 if z_i != 0 else 1

    x_seq = np.asarray(x_seq, dtype=np.float32)
    s0 = np.asarray(s0, dtype=np.float32)

    # ---- shared (replicated) weight packing ----
    # pw: phase-1 lhsT blocks per matrix: (mat, i, j) at col ((mat*2+i)*4+j)*128
    mats = [np.asarray(B0_w), np.asarray(B1_w), np.asarray(gate_w)][:NMAT] \
        if z_i != 0 else [np.asarray(B0_w), np.asarray(gate_w)]
    biases = [np.asarray(B0_b), np.asarray(B1_b), np.asarray(gate_b)][:NMAT] \
        if z_i != 0 else [np.asarray(B0_b), np.asarray(gate_b)]
    pw = np.concatenate(
        [_pack_lhsT_blocks(W.astype(np.float32), 2, 4, np.float32).reshape(128, 2, 4 * 128).reshape(128, -1)
         for W in mats], axis=1)
    # note: _pack_lhsT_blocks already gives (i*4+j) ordering per matrix
    pw = np.ascontiguousarray(pw)

    bias = np.zeros((128, 4 * NMAT), np.float32)
    for mi, bvec in enumerate(biases):
        bias[:, mi * 4:(mi + 1) * 4] = bvec.astype(np.float32).reshape(4, 128).T

    recs = [np.asarray(A0_w)] if z_i == 0 else [np.asarray(A0_w), np.asarray(A1_w)]
    aw = np.concatenate(
        [_pack_lhsT_blocks(A.astype(np.float32), 4, 4, np.float32)
         for A in recs], axis=1).astype(np.float16)
    aw = np.ascontiguousarray(aw)

    IDEN = np.ascontiguousarray(np.eye(128, dtype=np.float16))

    # ---- per-core inputs ----
    in_maps = []
    for c in range(N_CORES):
        bc = c * B_LOC
        xc = x_seq[bc:bc + B_LOC, :T]                       # [16, T, 256]
        xT = np.ascontiguousarray(
            xc.transpose(2, 1, 0).reshape(2, 128, T * B_LOC))
        s0c = s0[bc:bc + B_LOC]                             # [16, 512]
        s0T = np.ascontiguousarray(
            s0c.T.reshape(4, 128, B_LOC).transpose(1, 0, 2).reshape(128, 64))
        in_maps.append({
            "xT": xT, "pw": pw, "bias": bias, "aw": aw, "s0T": s0T,
            "iden": IDEN,
        })

    res = run_bass_kernel_spmd(nc, in_maps, list(range(N_CORES)), trace=_trace)
    if _trace:
        kernel._last_res = res

    out = np.empty((B_FULL, T + 1, S_DIM), np.float32)
    for c in range(N_CORES):
        bc = c * B_LOC
        stg = res.results[c]["stg"]                         # [T, 128, 64]
        out[bc:bc + B_LOC, 0] = s0[bc:bc + B_LOC]
        out[bc:bc + B_LOC, 1:] = (
            stg.reshape(T, 128, 4, B_LOC).transpose(3, 0, 2, 1)
            .reshape(B_LOC, T, S_DIM))
    return out



# revision 7
# speedup vs baseline: 1.3287x; 1.3287x over previous
"""Trainium2 Bass kernel for the BinaryMechanismSSM problem.

Full inputs in, full outputs out. Batch (128) sharded 8 ways (16/core).

Per core:
  Phase 1: projections bx0/bx1/gx = x @ {B0,B1,G}^T + bias as fp16 matmuls
           over 512-token tiles. bx planes are staged to DRAM pre-scaled by
           SCALE=512 (bias folded in); the gate plane is sigmoid(gx) fp16.
  Phase 2: T sequential steps. State held as st[p, 16j+b] = s[b, 128j+p]
           (fp16 [128, 64] tile). A0/A1 are fp8e4 (scaled by SCALE) with
           fp16 rhs; per step 32 A-matmuls + 4 identity-injection matmuls
           accumulate into 4 PSUM quarter tiles (one per state chunk j).
           MM issue order (iden, k=0 blocks, k=1 blocks, then per-j k=2/3
           blocks) lets the per-chunk tanh (scale=1/SCALE) + gate blend
           (DVE, fp16) pipeline underneath the matmuls of later chunks and
           of the next step, keeping the PE continuously busy (HAM-warm).
           States staged out fp16 every 4 steps; host re-layouts to
           [B, T+1, S] fp32.
"""
import numpy as np

B_FULL = 128
T_FULL = 1024
I_DIM = 256
S_DIM = 512
N_CORES = 8
B_LOC = B_FULL // N_CORES  # 16
SCALE = 512.0

_cache = {}


def _build(alpha: float, z: int, T: int):
    import ml_dtypes  # noqa: F401  (ensures fp8 numpy dtypes exist)
    import concourse.bass as bass  # noqa: F401
    from concourse import bacc
    import concourse.mybir as mybir
    from concourse.tile import TileContext

    dt = mybir.dt
    AF = mybir.ActivationFunctionType
    ALU = mybir.AluOpType

    TOK = T * B_LOC          # tokens per core
    NTT = TOK // 512         # phase-1 token tiles
    NG = T // 16             # phase-2 step groups
    NMAT = 3 if z != 0 else 2          # number of projection matrices
    NREC = 2 if z != 0 else 1          # number of recurrence matrices
    W2 = NREC * 16           # psum quarter width (m, b)

    nc = bacc.Bacc("TRN2", target_bir_lowering=False, debug=False,
                   num_devices=N_CORES)

    xT_d = nc.declare_dram_parameter("xT", [2, 128, TOK], dt.float16, isOutput=False)
    pw_d = nc.declare_dram_parameter("pw", [128, NMAT * 2 * 4 * 128], dt.float16, isOutput=False)
    bias_d = nc.declare_dram_parameter("bias", [128, 4 * NMAT], dt.float32, isOutput=False)
    aw_d = nc.declare_dram_parameter("aw", [128, NREC * 16 * 128], dt.float8e4, isOutput=False)
    s0_d = nc.declare_dram_parameter("s0T", [128, 64], dt.float16, isOutput=False)
    iden_d = nc.declare_dram_parameter("iden", [128, 128], dt.float8e4, isOutput=False)
    stg_d = nc.declare_dram_parameter("stg", [128, T, 64], dt.float16, isOutput=True)

    with TileContext(nc) as tc:
      with tc.tile_pool(name="dram", bufs=1, space="DRAM") as dpool:
        projb_p = [[dpool.tile([128, TOK], dt.float16, tag=f"projb{m}{j}",
                               name=f"projb{m}{j}")
                    for j in range(4)] for m in range(NREC)]
        projg_p = [dpool.tile([128, TOK], dt.float16, tag=f"projg{j}",
                              name=f"projg{j}") for j in range(4)]
        # ---------------- Phase 1: projections ----------------
        with (
            tc.tile_pool(name="p1w", bufs=1) as p1w,
            tc.tile_pool(name="p1x", bufs=3) as p1x,
            tc.tile_pool(name="p1o", bufs=6) as p1o,
            tc.tile_pool(name="p1ps", bufs=8, space="PSUM") as p1ps,
        ):
            pw = p1w.tile([128, NMAT * 2 * 4 * 128], dt.float16)
            nc.sync.dma_start(pw[:], pw_d[:])
            bias = p1w.tile([128, 4 * NMAT], dt.float32)
            nc.sync.dma_start(bias[:], bias_d[:])

            for tt in range(NTT):
                xt = p1x.tile([128, 2 * 512], dt.float16, tag="xt")
                for i in range(2):
                    nc.sync.dma_start(xt[:, i * 512:(i + 1) * 512],
                                      xT_d[i, :, tt * 512:(tt + 1) * 512])
                for mat in range(NMAT):
                    for j in range(4):
                        ps = p1ps.tile([128, 512], dt.float32, tag="pps")
                        for i in range(2):
                            blk = ((mat * 2 + i) * 4 + j) * 128
                            nc.tensor.matmul(
                                ps[:], pw[:, blk:blk + 128],
                                xt[:, i * 512:(i + 1) * 512],
                                start=(i == 0), stop=(i == 1))
                        bj = bias[:, mat * 4 + j:mat * 4 + j + 1]
                        ot = p1o.tile([128, 512], dt.float16, tag="po")
                        if mat == NMAT - 1:
                            nc.scalar.activation(ot[:], ps[:], AF.Sigmoid,
                                                 bias=bj, scale=1.0)
                            nc.sync.dma_start(
                                projg_p[j][:, tt * 512:(tt + 1) * 512], ot[:])
                        else:
                            # bias_d already holds SCALE*b for these mats
                            nc.scalar.activation(ot[:], ps[:], AF.Identity,
                                                 bias=bj, scale=SCALE)
                            nc.sync.dma_start(
                                projb_p[mat][j][:, tt * 512:(tt + 1) * 512],
                                ot[:])

        # ---------------- Phase 2: recurrence ----------------
        with (
            tc.tile_pool(name="p2w", bufs=1) as p2w,
            tc.tile_pool(name="p2in", bufs=2) as p2in,
            tc.tile_pool(name="p2st", bufs=2) as p2st,
            tc.tile_pool(name="p2c", bufs=2) as p2c,
            tc.tile_pool(name="p2ps", bufs=2, space="PSUM") as p2ps,
        ):
            aw = p2w.tile([128, NREC * 16 * 128], dt.float8e4)
            nc.sync.dma_start(aw[:], aw_d[:])
            iden = p2w.tile([128, 128], dt.float8e4)
            nc.sync.dma_start(iden[:], iden_d[:])
            s0t = p2w.tile([128, 64], dt.float16)
            nc.sync.dma_start(s0t[:], s0_d[:])

            a0 = float(1.0 - alpha) if z != 0 else 1.0
            a1 = float(alpha)
            NH = 2 if NREC == 2 else 1  # DVE half-splits

            st_prev = s0t
            obuf = None
            for g in range(NG):
                # fp16 staged projections: pjb cols = (m, j, t, b)
                pjb = p2in.tile([128, NREC * 4 * 256], dt.float16, tag="pjb")
                for m in range(NREC):
                    for j in range(4):
                        nc.sync.dma_start(
                            pjb[:, (m * 4 + j) * 256:(m * 4 + j + 1) * 256],
                            projb_p[m][j][:, g * 256:(g + 1) * 256])
                pjb_r = pjb[:].rearrange("p (m j t b) -> p m j t b",
                                         m=NREC, j=4, t=16, b=16)
                # gate plane (fp16), cols = (j, t, b)
                pjg = p2in.tile([128, 1024], dt.float16, tag="pjg")
                for j in range(4):
                    nc.sync.dma_start(
                        pjg[:, j * 256:(j + 1) * 256],
                        projg_p[j][:, g * 256:(g + 1) * 256])

                # per-group gate coefficient planes (off the serial path)
                gco = p2in.tile([128, NREC * 1024], dt.float16, tag="gco")
                nc.vector.tensor_scalar_mul(gco[:, 0:1024], pjg[:], a0)
                if NREC == 2:
                    nc.vector.tensor_scalar_mul(gco[:, 1024:2048], pjg[:], a1)
                # gco dims permuted to (j, m, t, b) for the half-wide DVE ops
                gco_r = gco[:].rearrange("p (m j t b) -> p j m t b",
                                         m=NREC, j=4, t=16, b=16)
                g1m = p2in.tile([128, 1024], dt.float16, tag="g1m")
                nc.vector.tensor_scalar(g1m[:], pjg[:], -1.0, 1.0,
                                        ALU.mult, ALU.add)
                g1mr = g1m[:].rearrange("p (j t b) -> p j t b", j=4, t=16)

                for tt in range(16):
                    t = g * 16 + tt
                    # m2 = (1-g) * s  (off serial path, only needs st_prev)
                    m2 = p2c.tile([128, 64], dt.float16, tag="m2")
                    nc.vector.tensor_tensor(
                        m2[:].rearrange("p (j b) -> p j b", j=4),
                        st_prev[:].rearrange("p (j b) -> p j b", j=4),
                        g1mr[:, :, tt, :], ALU.mult)

                    # 4 psum quarters, one per output state chunk j
                    pss = [p2ps.tile([128, W2], dt.float32, tag=f"ps{j}",
                                     name=f"ps{j}")
                           for j in range(4)]
                    # inject bx via fp8-identity matmuls (rhs fp16, exact)
                    for j in range(4):
                        nc.tensor.matmul(
                            pss[j][:].rearrange("p (m b) -> p m b", m=NREC),
                            iden[:], pjb_r[:, :, j, tt, :],
                            start=True, stop=False)
                    # k = 0, 1 A-blocks (all m, j): rhs ready earliest
                    for k in range(2):
                        for m in range(NREC):
                            for j in range(4):
                                blk = (m * 16 + k * 4 + j) * 128
                                nc.tensor.matmul(
                                    pss[j][:, m * 16:(m + 1) * 16],
                                    aw[:, blk:blk + 128],
                                    st_prev[:, k * 16:(k + 1) * 16],
                                    start=False, stop=False)

                    # new state tile: slice of the 4-step output buffer
                    if tt % 4 == 0:
                        obuf = p2st.tile([128, 4 * 64], dt.float16,
                                         tag="obuf")
                    st_new = obuf[:, (tt % 4) * 64:(tt % 4) * 64 + 64]

                    ft = p2c.tile([128, NREC * 64], dt.float16, tag="ft")
                    # per-j: finish contraction (k=2,3), then tanh
                    for j in range(4):
                        for k in range(2, 4):
                            for m in range(NREC):
                                blk = (m * 16 + k * 4 + j) * 128
                                nc.tensor.matmul(
                                    pss[j][:, m * 16:(m + 1) * 16],
                                    aw[:, blk:blk + 128],
                                    st_prev[:, k * 16:(k + 1) * 16],
                                    start=False,
                                    stop=(k == 3 and m == NREC - 1))
                        nc.scalar.activation(ft[:, j * W2:(j + 1) * W2],
                                             pss[j][:], AF.Tanh,
                                             scale=1.0 / SCALE)

                    # DVE gate blend at half (j-pair) granularity
                    for h in range(2):
                        fh = ft[:, h * 2 * W2:(h + 1) * 2 * W2]
                        if NREC == 2:
                            u = p2c.tile([128, 64], dt.float16, tag=f"u{h}")
                            nc.vector.tensor_tensor(
                                u[:].rearrange("p (j m b) -> p j m b",
                                               j=2, m=2),
                                fh.rearrange("p (j m b) -> p j m b",
                                             j=2, m=2),
                                gco_r[:, 2 * h:2 * h + 2, :, tt, :],
                                ALU.mult)
                            ur = u[:].rearrange("p (j m b) -> p j m b",
                                                j=2, m=2)
                            v = p2c.tile([128, 32], dt.float16, tag=f"v{h}")
                            nc.vector.tensor_tensor(
                                v[:].rearrange("p (j b) -> p j b", j=2),
                                ur[:, :, 0, :], ur[:, :, 1, :], ALU.add)
                        else:
                            v = p2c.tile([128, 32], dt.float16, tag=f"v{h}")
                            nc.vector.tensor_tensor(
                                v[:].rearrange("p (j b) -> p j b", j=2),
                                fh.rearrange("p (j m b) -> p j m b",
                                             j=2, m=1)[:, :, 0, :],
                                gco_r[:, 2 * h:2 * h + 2, 0, tt, :],
                                ALU.mult)
                        nc.vector.tensor_tensor(
                            st_new[:, h * 32:(h + 1) * 32], v[:],
                            m2[:, h * 32:(h + 1) * 32], ALU.add)

                    st_prev = st_new
                    if tt % 4 == 3:
                        nc.sync.dma_start(
                            stg_d[:, t - 3:t + 1, :],
                            obuf[:].rearrange("p (t c) -> p t c", t=4))

    nc.compile()
    return nc


def _pack_lhsT_blocks(W, kdim, mdim, dtype):
    """W: [mdim*128, kdim*128]; returns [128, kdim*mdim*128] with block
    (k, j) at cols (k*mdim+j)*128 equal to W[j-chunk, k-chunk].T."""
    nk, nj = kdim, mdim
    out = np.zeros((128, nk * nj * 128), dtype=dtype)
    for k in range(nk):
        for j in range(nj):
            blk = W[j * 128:(j + 1) * 128, k * 128:(k + 1) * 128].T
            out[:, (k * nj + j) * 128:(k * nj + j + 1) * 128] = blk
    return np.ascontiguousarray(out)


def kernel(x_seq, s0, A0_w, B0_w, B0_b, A1_w, B1_w, B1_b, gate_w, gate_b,
           alpha, z, _T=None, _trace=False):
    import ml_dtypes
    from concourse.bass_utils import run_bass_kernel_spmd

    T = int(_T or T_FULL)
    alpha_f = float(np.asarray(alpha))
    z_i = int(np.asarray(z))

    key = (alpha_f, z_i, T)
    if key not in _cache:
        _cache[key] = _build(alpha_f, z_i, T)
    nc = _cache[key]

    NMAT = 3 if z_i != 0 else 2
    NREC = 2 if z_i != 0 else 1

    x_seq = np.asarray(x_seq, dtype=np.float32)
    s0 = np.asarray(s0, dtype=np.float32)

    # ---- shared (replicated) weight packing ----
    mats = [np.asarray(B0_w), np.asarray(B1_w), np.asarray(gate_w)][:NMAT] \
        if z_i != 0 else [np.asarray(B0_w), np.asarray(gate_w)]
    biases = [np.asarray(B0_b), np.asarray(B1_b), np.asarray(gate_b)][:NMAT] \
        if z_i != 0 else [np.asarray(B0_b), np.asarray(gate_b)]
    pw = np.concatenate(
        [_pack_lhsT_blocks(W.astype(np.float32), 2, 4, np.float32)
         for W in mats], axis=1).astype(np.float16)
    pw = np.ascontiguousarray(pw)

    # bias for the bx mats is pre-scaled by SCALE (folded into phase-1 ACT)
    bias = np.zeros((128, 4 * NMAT), np.float32)
    for mi, bvec in enumerate(biases):
        scl = 1.0 if mi == NMAT - 1 else SCALE
        bias[:, mi * 4:(mi + 1) * 4] = \
            (scl * bvec.astype(np.float32)).reshape(4, 128).T

    recs = [np.asarray(A0_w)] if z_i == 0 else [np.asarray(A0_w), np.asarray(A1_w)]
    aw = np.concatenate(
        [_pack_lhsT_blocks(A.astype(np.float32), 4, 4, np.float32)
         for A in recs], axis=1) * SCALE
    aw = np.ascontiguousarray(np.clip(aw, -240.0, 240.0)).astype(
        ml_dtypes.float8_e4m3)

    IDEN = np.ascontiguousarray(np.eye(128).astype(ml_dtypes.float8_e4m3))

    # ---- per-core inputs ----
    in_maps = []
    for c in range(N_CORES):
        bc = c * B_LOC
        xc = x_seq[bc:bc + B_LOC, :T]                       # [16, T, 256]
        xT = np.ascontiguousarray(
            xc.transpose(2, 1, 0).reshape(2, 128, T * B_LOC)).astype(
                np.float16)
        s0c = s0[bc:bc + B_LOC]                             # [16, 512]
        s0T = np.ascontiguousarray(
            s0c.T.reshape(4, 128, B_LOC).transpose(1, 0, 2).reshape(128, 64)
        ).astype(np.float16)
        in_maps.append({
            "xT": xT, "pw": pw, "bias": bias, "aw": aw, "s0T": s0T,
            "iden": IDEN,
        })

    res = run_bass_kernel_spmd(nc, in_maps, list(range(N_CORES)), trace=_trace)
    if _trace:
        kernel._last_res = res

    out = np.empty((B_FULL, T + 1, S_DIM), np.float32)
    for c in range(N_CORES):
        bc = c * B_LOC
        stg = np.asarray(res.results[c]["stg"]).astype(np.float32)
        out[bc:bc + B_LOC, 0] = s0[bc:bc + B_LOC]
        out[bc:bc + B_LOC, 1:] = (
            stg.reshape(128, T, 4, B_LOC).transpose(3, 1, 2, 0)
            .reshape(B_LOC, T, S_DIM))
    return out


# revision 10
# speedup vs baseline: 1.3499x; 1.0159x over previous
"""Trainium2 Bass kernel for the BinaryMechanismSSM problem.

Full inputs in, full outputs out. Batch (128) sharded 8 ways (16/core).

Per core:
  Phase 1: projections bx0/bx1/gx = x @ {B0,B1,G}^T + bias as fp16 matmuls
           over 512-token tiles. bx planes are staged to DRAM pre-scaled by
           SCALE=512 (bias folded in); the gate plane is sigmoid(gx) fp16.
  Phase 2: T sequential steps. State held as st[p, 16j+b] = s[b, 128j+p]
           (fp16 [128, 64] tile). A0/A1 are fp8e4 (scaled by SCALE) with
           fp16 rhs; per step 32 A-matmuls + 4 identity-injection matmuls
           accumulate into 4 PSUM quarter tiles (one per state chunk j).
           MM issue order (iden, k=0 blocks, k=1 blocks, then per-j k=2/3
           blocks) lets the per-chunk tanh (scale=1/SCALE) + gate blend
           (DVE, fp16) pipeline underneath the matmuls of later chunks and
           of the next step, keeping the PE continuously busy (HAM-warm).
           States staged out fp16 every 4 steps; host re-layouts to
           [B, T+1, S] fp32.
"""
import numpy as np

B_FULL = 128
T_FULL = 1024
I_DIM = 256
S_DIM = 512
N_CORES = 8
B_LOC = B_FULL // N_CORES  # 16
SCALE = 512.0

_cache = {}


def _build(alpha: float, z: int, T: int):
    import ml_dtypes  # noqa: F401  (ensures fp8 numpy dtypes exist)
    import concourse.bass as bass  # noqa: F401
    from concourse import bacc
    import concourse.mybir as mybir
    from concourse.tile import TileContext

    dt = mybir.dt
    AF = mybir.ActivationFunctionType
    ALU = mybir.AluOpType

    TOK = T * B_LOC          # tokens per core
    NTT = TOK // 512         # phase-1 token tiles
    NG = T // 16             # phase-2 step groups
    NMAT = 3 if z != 0 else 2          # number of projection matrices
    NREC = 2 if z != 0 else 1          # number of recurrence matrices
    W2 = NREC * 16           # psum quarter width (m, b)

    nc = bacc.Bacc("TRN2", target_bir_lowering=False, debug=False,
                   num_devices=N_CORES)

    xT_d = nc.declare_dram_parameter("xT", [2, 128, TOK], dt.float16, isOutput=False)
    pw_d = nc.declare_dram_parameter("pw", [128, NMAT * 2 * 4 * 128], dt.float16, isOutput=False)
    bias_d = nc.declare_dram_parameter("bias", [128, 4 * NMAT], dt.float32, isOutput=False)
    aw_d = nc.declare_dram_parameter("aw", [128, NREC * 16 * 128], dt.float8e4, isOutput=False)
    s0_d = nc.declare_dram_parameter("s0T", [128, 64], dt.float16, isOutput=False)
    iden_d = nc.declare_dram_parameter("iden", [128, 128], dt.float8e4, isOutput=False)
    stg_d = nc.declare_dram_parameter("stg", [128, T, 64], dt.float16, isOutput=True)

    with TileContext(nc) as tc:
      with tc.tile_pool(name="dram", bufs=1, space="DRAM") as dpool:
        projb_p = [[dpool.tile([128, TOK], dt.float16, tag=f"projb{m}{j}",
                               name=f"projb{m}{j}")
                    for j in range(4)] for m in range(NREC)]
        projg_p = [dpool.tile([128, TOK], dt.float16, tag=f"projg{j}",
                              name=f"projg{j}") for j in range(4)]
        # ---------------- Phase 1: projections ----------------
        with (
            tc.tile_pool(name="p1w", bufs=1) as p1w,
            tc.tile_pool(name="p1x", bufs=3) as p1x,
            tc.tile_pool(name="p1o", bufs=6) as p1o,
            tc.tile_pool(name="p1ps", bufs=8, space="PSUM") as p1ps,
        ):
            pw = p1w.tile([128, NMAT * 2 * 4 * 128], dt.float16)
            nc.sync.dma_start(pw[:], pw_d[:])
            bias = p1w.tile([128, 4 * NMAT], dt.float32)
            nc.sync.dma_start(bias[:], bias_d[:])

            for tt in range(NTT):
                xt = p1x.tile([128, 2 * 512], dt.float16, tag="xt")
                for i in range(2):
                    nc.sync.dma_start(xt[:, i * 512:(i + 1) * 512],
                                      xT_d[i, :, tt * 512:(tt + 1) * 512])
                for mat in range(NMAT):
                    for j in range(4):
                        ps = p1ps.tile([128, 512], dt.float32, tag="pps")
                        for i in range(2):
                            blk = ((mat * 2 + i) * 4 + j) * 128
                            nc.tensor.matmul(
                                ps[:], pw[:, blk:blk + 128],
                                xt[:, i * 512:(i + 1) * 512],
                                start=(i == 0), stop=(i == 1))
                        bj = bias[:, mat * 4 + j:mat * 4 + j + 1]
                        ot = p1o.tile([128, 512], dt.float16, tag="po")
                        if mat == NMAT - 1:
                            nc.scalar.activation(ot[:], ps[:], AF.Sigmoid,
                                                 bias=bj, scale=1.0)
                            nc.sync.dma_start(
                                projg_p[j][:, tt * 512:(tt + 1) * 512], ot[:])
                        else:
                            # bias_d already holds SCALE*b for these mats
                            nc.scalar.activation(ot[:], ps[:], AF.Identity,
                                                 bias=bj, scale=SCALE)
                            nc.sync.dma_start(
                                projb_p[mat][j][:, tt * 512:(tt + 1) * 512],
                                ot[:])

        # ---------------- Phase 2: recurrence ----------------
        with (
            tc.tile_pool(name="p2w", bufs=1) as p2w,
            tc.tile_pool(name="p2in", bufs=2) as p2in,
            tc.tile_pool(name="p2st", bufs=2) as p2st,
            tc.tile_pool(name="p2c", bufs=2) as p2c,
            tc.tile_pool(name="p2ps", bufs=4, space="PSUM") as p2ps,
        ):
            aw = p2w.tile([128, NREC * 16 * 128], dt.float8e4)
            nc.sync.dma_start(aw[:], aw_d[:])
            iden = p2w.tile([128, 128], dt.float8e4)
            nc.sync.dma_start(iden[:], iden_d[:])
            s0t = p2w.tile([128, 64], dt.float16)
            nc.sync.dma_start(s0t[:], s0_d[:])

            a0 = float(1.0 - alpha) if z != 0 else 1.0
            a1 = float(alpha)
            WH = NREC * 32            # psum half width: (m, j2, b)

            st_prev = s0t
            obuf = None
            for g in range(NG):
                # fp16 staged projections: pjb cols = (m, j, t, b)
                pjb = p2in.tile([128, NREC * 4 * 256], dt.float16, tag="pjb")
                for m in range(NREC):
                    for j in range(4):
                        nc.sync.dma_start(
                            pjb[:, (m * 4 + j) * 256:(m * 4 + j + 1) * 256],
                            projb_p[m][j][:, g * 256:(g + 1) * 256])
                pjb_r = pjb[:].rearrange("p (m j t b) -> p m j t b",
                                         m=NREC, j=4, t=16, b=16)
                # gate plane (fp16), cols = (j, t, b)
                pjg = p2in.tile([128, 1024], dt.float16, tag="pjg")
                for j in range(4):
                    nc.sync.dma_start(
                        pjg[:, j * 256:(j + 1) * 256],
                        projg_p[j][:, g * 256:(g + 1) * 256])

                # per-group gate coefficient planes (off the serial path)
                gco = p2in.tile([128, NREC * 1024], dt.float16, tag="gco")
                nc.vector.tensor_scalar_mul(gco[:, 0:1024], pjg[:], a0)
                if NREC == 2:
                    nc.vector.tensor_scalar_mul(gco[:, 1024:2048], pjg[:], a1)
                gco_r = gco[:].rearrange("p (m j t b) -> p m j t b",
                                         m=NREC, j=4, t=16, b=16)
                g1m = p2in.tile([128, 1024], dt.float16, tag="g1m")
                nc.vector.tensor_scalar(g1m[:], pjg[:], -1.0, 1.0,
                                        ALU.mult, ALU.add)
                g1mr = g1m[:].rearrange("p (j t b) -> p j t b", j=4, t=16)

                for tt in range(16):
                    t = g * 16 + tt
                    # m2 = (1-g) * s  (off serial path, only needs st_prev)
                    m2 = p2c.tile([128, 64], dt.float16, tag="m2")
                    nc.vector.tensor_tensor(
                        m2[:].rearrange("p (j b) -> p j b", j=4),
                        st_prev[:].rearrange("p (j b) -> p j b", j=4),
                        g1mr[:, :, tt, :], ALU.mult)

                    # 2 psum halves; cols = (m, j2, b) for half j-pair
                    pss = [p2ps.tile([128, WH], dt.float32, tag=f"ps{h}",
                                     name=f"ps{h}")
                           for h in range(2)]
                    # inject bx via fp8-identity matmuls (rhs fp16, exact)
                    for h in range(2):
                        nc.tensor.matmul(
                            pss[h][:].rearrange("p (m j b) -> p m j b",
                                                m=NREC, j=2),
                            iden[:], pjb_r[:, :, 2 * h:2 * h + 2, tt, :],
                            start=True, stop=False)

                    def a_mm(h, k, last=False):
                        for m in range(NREC):
                            for jj in range(2):
                                j = 2 * h + jj
                                blk = (m * 16 + k * 4 + j) * 128
                                nc.tensor.matmul(
                                    pss[h][:, m * 32 + jj * 16:
                                           m * 32 + jj * 16 + 16],
                                    aw[:, blk:blk + 128],
                                    st_prev[:, k * 16:(k + 1) * 16],
                                    start=False,
                                    stop=(last and m == NREC - 1
                                          and jj == 1))

                    # k = 0,1 for both halves (rhs chunks ready earliest)
                    a_mm(0, 0)
                    a_mm(1, 0)
                    a_mm(0, 1)
                    a_mm(1, 1)

                    # new state tile: slice of the 4-step output buffer
                    if tt % 4 == 0:
                        obuf = p2st.tile([128, 4 * 64], dt.float16,
                                         tag="obuf")
                    st_new = obuf[:, (tt % 4) * 64:(tt % 4) * 64 + 64]

                    ft = p2c.tile([128, 2 * WH], dt.float16, tag="ft")
                    for h in range(2):
                        a_mm(h, 2)
                        a_mm(h, 3, last=True)
                        nc.scalar.activation(ft[:, h * WH:(h + 1) * WH],
                                             pss[h][:], AF.Tanh,
                                             scale=1.0 / SCALE)

                    # DVE gate blend per half: u = ft*gco, v = sum_m u,
                    # st = v + m2
                    for h in range(2):
                        fh = ft[:, h * WH:(h + 1) * WH]
                        gslice = gco_r[:, :, 2 * h:2 * h + 2, tt, :]
                        if NREC == 2:
                            u = p2c.tile([128, 64], dt.float16, tag=f"u{h}")
                            ur = u[:].rearrange("p (m j b) -> p m j b",
                                                m=2, j=2)
                            nc.vector.tensor_tensor(
                                ur, fh.rearrange("p (m j b) -> p m j b",
                                                 m=2, j=2),
                                gslice, ALU.mult)
                            v = p2c.tile([128, 32], dt.float16, tag=f"v{h}")
                            nc.vector.tensor_tensor(
                                v[:].rearrange("p (j b) -> p j b", j=2),
                                ur[:, 0], ur[:, 1], ALU.add)
                        else:
                            v = p2c.tile([128, 32], dt.float16, tag=f"v{h}")
                            nc.vector.tensor_tensor(
                                v[:].rearrange("p (j b) -> p j b", j=2),
                                fh.rearrange("p (m j b) -> p m j b",
                                             m=1, j=2)[:, 0],
                                gslice[:, 0], ALU.mult)
                        nc.vector.tensor_tensor(
                            st_new[:, h * 32:(h + 1) * 32], v[:],
                            m2[:, h * 32:(h + 1) * 32], ALU.add)

                    st_prev = st_new
                    if tt % 4 == 3:
                        nc.sync.dma_start(
                            stg_d[:, t - 3:t + 1, :],
                            obuf[:].rearrange("p (t c) -> p t c", t=4))

    nc.compile()
    return nc


def _pack_lhsT_blocks(W, kdim, mdim, dtype):
    """W: [mdim*128, kdim*128]; returns [128, kdim*mdim*128] with block
    (k, j) at cols (k*mdim+j)*128 equal to W[j-chunk, k-chunk].T."""
    nk, nj = kdim, mdim
    out = np.zeros((128, nk * nj * 128), dtype=dtype)
    for k in range(nk):
        for j in range(nj):
            blk = W[j * 128:(j + 1) * 128, k * 128:(k + 1) * 128].T
            out[:, (k * nj + j) * 128:(k * nj + j + 1) * 128] = blk
    return np.ascontiguousarray(out)


def kernel(x_seq, s0, A0_w, B0_w, B0_b, A1_w, B1_w, B1_b, gate_w, gate_b,
           alpha, z, _T=None, _trace=False):
    import ml_dtypes
    from concourse.bass_utils import run_bass_kernel_spmd

    T = int(_T or T_FULL)
    alpha_f = float(np.asarray(alpha))
    z_i = int(np.asarray(z))

    key = (alpha_f, z_i, T)
    if key not in _cache:
        _cache[key] = _build(alpha_f, z_i, T)
    nc = _cache[key]

    NMAT = 3 if z_i != 0 else 2
    NREC = 2 if z_i != 0 else 1

    x_seq = np.asarray(x_seq, dtype=np.float32)
    s0 = np.asarray(s0, dtype=np.float32)

    # ---- shared (replicated) weight packing ----
    mats = [np.asarray(B0_w), np.asarray(B1_w), np.asarray(gate_w)][:NMAT] \
        if z_i != 0 else [np.asarray(B0_w), np.asarray(gate_w)]
    biases = [np.asarray(B0_b), np.asarray(B1_b), np.asarray(gate_b)][:NMAT] \
        if z_i != 0 else [np.asarray(B0_b), np.asarray(gate_b)]
    pw = np.concatenate(
        [_pack_lhsT_blocks(W.astype(np.float32), 2, 4, np.float32)
         for W in mats], axis=1).astype(np.float16)
    pw = np.ascontiguousarray(pw)

    # bias for the bx mats is pre-scaled by SCALE (folded into phase-1 ACT)
    bias = np.zeros((128, 4 * NMAT), np.float32)
    for mi, bvec in enumerate(biases):
        scl = 1.0 if mi == NMAT - 1 else SCALE
        bias[:, mi * 4:(mi + 1) * 4] = \
            (scl * bvec.astype(np.float32)).reshape(4, 128).T

    recs = [np.asarray(A0_w)] if z_i == 0 else [np.asarray(A0_w), np.asarray(A1_w)]
    aw = np.concatenate(
        [_pack_lhsT_blocks(A.astype(np.float32), 4, 4, np.float32)
         for A in recs], axis=1) * SCALE
    aw = np.ascontiguousarray(np.clip(aw, -240.0, 240.0)).astype(
        ml_dtypes.float8_e4m3)

    IDEN = np.ascontiguousarray(np.eye(128).astype(ml_dtypes.float8_e4m3))

    # ---- per-core inputs ----
    in_maps = []
    for c in range(N_CORES):
        bc = c * B_LOC
        xc = x_seq[bc:bc + B_LOC, :T]                       # [16, T, 256]
        xT = np.ascontiguousarray(
            xc.transpose(2, 1, 0).reshape(2, 128, T * B_LOC)).astype(
                np.float16)
        s0c = s0[bc:bc + B_LOC]                             # [16, 512]
        s0T = np.ascontiguousarray(
            s0c.T.reshape(4, 128, B_LOC).transpose(1, 0, 2).reshape(128, 64)
        ).astype(np.float16)
        in_maps.append({
            "xT": xT, "pw": pw, "bias": bias, "aw": aw, "s0T": s0T,
            "iden": IDEN,
        })

    res = run_bass_kernel_spmd(nc, in_maps, list(range(N_CORES)), trace=_trace)
    if _trace:
        kernel._last_res = res

    out = np.empty((B_FULL, T + 1, S_DIM), np.float32)
    for c in range(N_CORES):
        bc = c * B_LOC
        stg = np.asarray(res.results[c]["stg"]).astype(np.float32)
        out[bc:bc + B_LOC, 0] = s0[bc:bc + B_LOC]
        out[bc:bc + B_LOC, 1:] = (
            stg.reshape(128, T, 4, B_LOC).transpose(3, 1, 2, 0)
            .reshape(B_LOC, T, S_DIM))
    return out


# revision 13
# speedup vs baseline: 1.3777x; 1.0206x over previous
"""Trainium2 Bass kernel for the BinaryMechanismSSM problem.

Full inputs in, full outputs out. Batch (128) sharded 8 ways (16/core).

Per core:
  Phase 1: projections bx0/bx1/gx = x @ {B0,B1,G}^T + bias as fp16 matmuls
           over 512-token tiles. bx planes are staged to DRAM pre-scaled by
           SCALE=512 (bias folded in); the gate plane is sigmoid(gx) fp16.
  Phase 2: T sequential steps. State held as st[p, 16j+b] = s[b, 128j+p]
           (fp16 [128, 64] tile). A0/A1 are fp8e4 (scaled by SCALE) with
           fp16 rhs; per step 32 A-matmuls + 4 identity-injection matmuls
           accumulate into 4 PSUM quarter tiles (one per state chunk j).
           MM issue order (iden, k=0 blocks, k=1 blocks, then per-j k=2/3
           blocks) lets the per-chunk tanh (scale=1/SCALE) + gate blend
           (DVE, fp16) pipeline underneath the matmuls of later chunks and
           of the next step, keeping the PE continuously busy (HAM-warm).
           States staged out fp16 every 4 steps; host re-layouts to
           [B, T+1, S] fp32.
"""
import numpy as np

B_FULL = 128
T_FULL = 1024
I_DIM = 256
S_DIM = 512
N_CORES = 8
B_LOC = B_FULL // N_CORES  # 16
SCALE = 512.0

_cache = {}


def _build(alpha: float, z: int, T: int):
    import ml_dtypes  # noqa: F401  (ensures fp8 numpy dtypes exist)
    import concourse.bass as bass  # noqa: F401
    from concourse import bacc
    import concourse.mybir as mybir
    from concourse.tile import TileContext

    dt = mybir.dt
    AF = mybir.ActivationFunctionType
    ALU = mybir.AluOpType

    TOK = T * B_LOC          # tokens per core
    NTT = TOK // 512         # phase-1 token tiles
    NG = T // 16             # phase-2 step groups
    NMAT = 3 if z != 0 else 2          # number of projection matrices
    NREC = 2 if z != 0 else 1          # number of recurrence matrices
    W2 = NREC * 16           # psum quarter width (m, b)

    nc = bacc.Bacc("TRN2", target_bir_lowering=False, debug=False,
                   num_devices=N_CORES)

    xT_d = nc.declare_dram_parameter("xT", [2, 128, TOK], dt.float16, isOutput=False)
    pw_d = nc.declare_dram_parameter("pw", [128, NMAT * 2 * 4 * 128], dt.float16, isOutput=False)
    bias_d = nc.declare_dram_parameter("bias", [128, 4 * NMAT], dt.float32, isOutput=False)
    aw_d = nc.declare_dram_parameter("aw", [128, NREC * 16 * 128], dt.float8e4, isOutput=False)
    s0_d = nc.declare_dram_parameter("s0T", [128, 64], dt.float16, isOutput=False)
    iden_d = nc.declare_dram_parameter("iden", [128, 128], dt.float8e4, isOutput=False)
    stg_d = nc.declare_dram_parameter("stg", [128, T, 64], dt.float16, isOutput=True)

    with TileContext(nc) as tc:
      with tc.tile_pool(name="dram", bufs=1, space="DRAM") as dpool:
        projb_p = [[dpool.tile([128, TOK], dt.float16, tag=f"projb{m}{j}",
                               name=f"projb{m}{j}")
                    for j in range(4)] for m in range(NREC)]
        projg_p = [dpool.tile([128, TOK], dt.float16, tag=f"projg{j}",
                              name=f"projg{j}") for j in range(4)]
        # ---------------- Phase 1: projections ----------------
        with (
            tc.tile_pool(name="p1w", bufs=1) as p1w,
            tc.tile_pool(name="p1x", bufs=3) as p1x,
            tc.tile_pool(name="p1o", bufs=6) as p1o,
            tc.tile_pool(name="p1ps", bufs=8, space="PSUM") as p1ps,
        ):
            pw = p1w.tile([128, NMAT * 2 * 4 * 128], dt.float16)
            nc.sync.dma_start(pw[:], pw_d[:])
            bias = p1w.tile([128, 4 * NMAT], dt.float32)
            nc.sync.dma_start(bias[:], bias_d[:])

            for tt in range(NTT):
                xt = p1x.tile([128, 2 * 512], dt.float16, tag="xt")
                for i in range(2):
                    nc.sync.dma_start(xt[:, i * 512:(i + 1) * 512],
                                      xT_d[i, :, tt * 512:(tt + 1) * 512])
                for mat in range(NMAT):
                    for j in range(4):
                        ps = p1ps.tile([128, 512], dt.float32, tag="pps")
                        for i in range(2):
                            blk = ((mat * 2 + i) * 4 + j) * 128
                            nc.tensor.matmul(
                                ps[:], pw[:, blk:blk + 128],
                                xt[:, i * 512:(i + 1) * 512],
                                start=(i == 0), stop=(i == 1))
                        bj = bias[:, mat * 4 + j:mat * 4 + j + 1]
                        ot = p1o.tile([128, 512], dt.float16, tag="po")
                        if mat == NMAT - 1:
                            nc.scalar.activation(ot[:], ps[:], AF.Sigmoid,
                                                 bias=bj, scale=1.0)
                            nc.sync.dma_start(
                                projg_p[j][:, tt * 512:(tt + 1) * 512], ot[:])
                        else:
                            # bias_d already holds SCALE*b for these mats
                            nc.scalar.activation(ot[:], ps[:], AF.Identity,
                                                 bias=bj, scale=SCALE)
                            nc.sync.dma_start(
                                projb_p[mat][j][:, tt * 512:(tt + 1) * 512],
                                ot[:])

        # ---------------- Phase 2: recurrence ----------------
        with (
            tc.tile_pool(name="p2w", bufs=1) as p2w,
            tc.tile_pool(name="p2in", bufs=1) as p2in,
            tc.tile_pool(name="p2st", bufs=1) as p2st,
            tc.tile_pool(name="p2c", bufs=1) as p2c,
            tc.tile_pool(name="p2ps", bufs=1, space="PSUM") as p2ps,
        ):
            aw = p2w.tile([128, NREC * 16 * 128], dt.float8e4)
            nc.sync.dma_start(aw[:], aw_d[:])
            iden = p2w.tile([128, 128], dt.float8e4)
            nc.sync.dma_start(iden[:], iden_d[:])
            s0t = p2w.tile([128, 64], dt.float16)
            nc.sync.dma_start(s0t[:], s0_d[:])

            a0 = float(1.0 - alpha) if z != 0 else 1.0
            a1 = float(alpha)
            WH = NREC * 32            # psum half width: (m, j2, b)
            ME = NREC + 1             # extended m-dim: mats + carry slice

            # explicit rings (pool buf rotation serializes one generation
            # too tight on the hot path; rings decouple by construction)
            pss_r = [[p2ps.tile([128, WH], dt.float32, tag=f"ps{h}{r}",
                                name=f"ps{h}{r}") for r in range(4)]
                     for h in range(2)]
            ft_r = [p2c.tile([128, 2 * WH], dt.float16, tag=f"ft{r}",
                             name=f"ft{r}") for r in range(2)]
            u_r = [[p2c.tile([128, ME * 32], dt.float16, tag=f"u{h}{r}",
                             name=f"u{h}{r}") for r in range(2)]
                   for h in range(2)]
            ob_r = [p2st.tile([128, 4 * 64], dt.float16, tag=f"ob{r}",
                              name=f"ob{r}") for r in range(2)]
            pjb_r_ = [p2in.tile([128, NREC * 4 * 256], dt.float16,
                                tag=f"pjb{r}", name=f"pjb{r}")
                      for r in range(2)]
            pjg_r_ = [p2in.tile([128, 1024], dt.float16, tag=f"pjg{r}",
                                name=f"pjg{r}") for r in range(2)]
            gco_r_ = [p2in.tile([128, NREC * 1024], dt.float16,
                                tag=f"gco{r}", name=f"gco{r}")
                      for r in range(2)]
            g1m_r_ = [p2in.tile([128, 1024], dt.float16, tag=f"g1m{r}",
                                name=f"g1m{r}") for r in range(2)]

            def stage_group(g):
                """DMA-in staged planes + build gate coeffs for group g."""
                r = g % 2
                pjb, pjg = pjb_r_[r], pjg_r_[r]
                gco, g1m = gco_r_[r], g1m_r_[r]
                for m in range(NREC):
                    for j in range(4):
                        nc.sync.dma_start(
                            pjb[:, (m * 4 + j) * 256:(m * 4 + j + 1) * 256],
                            projb_p[m][j][:, g * 256:(g + 1) * 256])
                for j in range(4):
                    nc.sync.dma_start(
                        pjg[:, j * 256:(j + 1) * 256],
                        projg_p[j][:, g * 256:(g + 1) * 256])
                nc.vector.tensor_scalar_mul(gco[:, 0:1024], pjg[:], a0)
                if NREC == 2:
                    nc.vector.tensor_scalar_mul(gco[:, 1024:2048], pjg[:],
                                                a1)
                nc.vector.tensor_scalar(g1m[:], pjg[:], -1.0, 1.0,
                                        ALU.mult, ALU.add)

            stage_group(0)
            st_prev = s0t
            for g in range(NG):
                r = g % 2
                pjbr = pjb_r_[r][:].rearrange("p (m j t b) -> p m j t b",
                                              m=NREC, j=4, t=16, b=16)
                gcor = gco_r_[r][:].rearrange("p (m j t b) -> p m j t b",
                                              m=NREC, j=4, t=16, b=16)
                g1mr = g1m_r_[r][:].rearrange("p (j t b) -> p j t b",
                                              j=4, t=16)

                for tt in range(16):
                    t = g * 16 + tt
                    pss = [pss_r[h][t % 4] for h in range(2)]
                    ft = ft_r[t % 2]
                    uu = [u_r[h][t % 2] for h in range(2)]
                    obuf = ob_r[(t // 4) % 2]
                    st_new = obuf[:, (tt % 4) * 64:(tt % 4) * 64 + 64]

                    # carry slice: u[:, NREC*32:] = (1-g) * s  (off path)
                    for h in range(2):
                        nc.vector.tensor_tensor(
                            uu[h][:, NREC * 32:ME * 32]
                            .rearrange("p (j b) -> p j b", j=2),
                            st_prev[:, h * 32:(h + 1) * 32]
                            .rearrange("p (j b) -> p j b", j=2),
                            g1mr[:, 2 * h:2 * h + 2, tt, :], ALU.mult)

                    # inject bx via fp8-identity matmuls (rhs fp16, exact)
                    for h in range(2):
                        nc.tensor.matmul(
                            pss[h][:].rearrange("p (m j b) -> p m j b",
                                                m=NREC, j=2),
                            iden[:], pjbr[:, :, 2 * h:2 * h + 2, tt, :],
                            start=True, stop=False)

                    def a_mm(h, k, last=False):
                        for m in range(NREC):
                            for jj in range(2):
                                j = 2 * h + jj
                                blk = (m * 16 + k * 4 + j) * 128
                                nc.tensor.matmul(
                                    pss[h][:, m * 32 + jj * 16:
                                           m * 32 + jj * 16 + 16],
                                    aw[:, blk:blk + 128],
                                    st_prev[:, k * 16:(k + 1) * 16],
                                    start=False,
                                    stop=(last and m == NREC - 1
                                          and jj == 1))

                    # half A fully first (its chain gates next step's k0/k1)
                    a_mm(0, 0)
                    a_mm(0, 1)
                    a_mm(0, 2)
                    a_mm(0, 3, last=True)
                    nc.scalar.activation(ft[:, 0:WH], pss[0][:], AF.Tanh,
                                         scale=1.0 / SCALE)
                    a_mm(1, 0)
                    a_mm(1, 1)
                    a_mm(1, 2)
                    a_mm(1, 3, last=True)
                    nc.scalar.activation(ft[:, WH:2 * WH], pss[1][:],
                                         AF.Tanh, scale=1.0 / SCALE)

                    # group staging prefetch mid-step-loop (off the path)
                    if tt == 8 and g + 1 < NG:
                        stage_group(g + 1)

                    # DVE: u = ft*gco (mats), then st = reduce_m(u) with the
                    # carry slice folded in as the third m-slice
                    for h in range(2):
                        nc.vector.tensor_tensor(
                            uu[h][:, 0:NREC * 32]
                            .rearrange("p (m j b) -> p m j b", m=NREC, j=2),
                            ft[:, h * WH:(h + 1) * WH]
                            .rearrange("p (m j b) -> p m j b", m=NREC, j=2),
                            gcor[:, :, 2 * h:2 * h + 2, tt, :], ALU.mult)
                        with nc.allow_low_precision("fp16 3-term gate sum"):
                            nc.vector.tensor_reduce(
                                st_new[:, h * 32:(h + 1) * 32]
                                .rearrange("p (j b) -> p j b", j=2),
                                uu[h][:].rearrange("p (m j b) -> p j b m",
                                                   m=ME, j=2),
                                mybir.AxisListType.X, ALU.add)

                    st_prev = st_new
                    if tt % 4 == 3:
                        nc.sync.dma_start(
                            stg_d[:, t - 3:t + 1, :],
                            obuf[:].rearrange("p (t c) -> p t c", t=4))

    nc.compile()
    return nc


def _pack_lhsT_blocks(W, kdim, mdim, dtype):
    """W: [mdim*128, kdim*128]; returns [128, kdim*mdim*128] with block
    (k, j) at cols (k*mdim+j)*128 equal to W[j-chunk, k-chunk].T."""
    nk, nj = kdim, mdim
    out = np.zeros((128, nk * nj * 128), dtype=dtype)
    for k in range(nk):
        for j in range(nj):
            blk = W[j * 128:(j + 1) * 128, k * 128:(k + 1) * 128].T
            out[:, (k * nj + j) * 128:(k * nj + j + 1) * 128] = blk
    return np.ascontiguousarray(out)


def kernel(x_seq, s0, A0_w, B0_w, B0_b, A1_w, B1_w, B1_b, gate_w, gate_b,
           alpha, z, _T=None, _trace=False):
    import ml_dtypes
    from concourse.bass_utils import run_bass_kernel_spmd

    T = int(_T or T_FULL)
    alpha_f = float(np.asarray(alpha))
    z_i = int(np.asarray(z))

    key = (alpha_f, z_i, T)
    if key not in _cache:
        _cache[key] = _build(alpha_f, z_i, T)
    nc = _cache[key]

    NMAT = 3 if z_i != 0 else 2
    NREC = 2 if z_i != 0 else 1

    x_seq = np.asarray(x_seq, dtype=np.float32)
    s0 = np.asarray(s0, dtype=np.float32)

    # ---- shared (replicated) weight packing ----
    mats = [np.asarray(B0_w), np.asarray(B1_w), np.asarray(gate_w)][:NMAT] \
        if z_i != 0 else [np.asarray(B0_w), np.asarray(gate_w)]
    biases = [np.asarray(B0_b), np.asarray(B1_b), np.asarray(gate_b)][:NMAT] \
        if z_i != 0 else [np.asarray(B0_b), np.asarray(gate_b)]
    pw = np.concatenate(
        [_pack_lhsT_blocks(W.astype(np.float32), 2, 4, np.float32)
         for W in mats], axis=1).astype(np.float16)
    pw = np.ascontiguousarray(pw)

    # bias for the bx mats is pre-scaled by SCALE (folded into phase-1 ACT)
    bias = np.zeros((128, 4 * NMAT), np.float32)
    for mi, bvec in enumerate(biases):
        scl = 1.0 if mi == NMAT - 1 else SCALE
        bias[:, mi * 4:(mi + 1) * 4] = \
            (scl * bvec.astype(np.float32)).reshape(4, 128).T

    recs = [np.asarray(A0_w)] if z_i == 0 else [np.asarray(A0_w), np.asarray(A1_w)]
    aw = np.concatenate(
        [_pack_lhsT_blocks(A.astype(np.float32), 4, 4, np.float32)
         for A in recs], axis=1) * SCALE
    aw = np.ascontiguousarray(np.clip(aw, -240.0, 240.0)).astype(
        ml_dtypes.float8_e4m3)

    IDEN = np.ascontiguousarray(np.eye(128).astype(ml_dtypes.float8_e4m3))

    # ---- per-core inputs ----
    in_maps = []
    for c in range(N_CORES):
        bc = c * B_LOC
        xc = x_seq[bc:bc + B_LOC, :T]                       # [16, T, 256]
        xT = np.ascontiguousarray(
            xc.transpose(2, 1, 0).reshape(2, 128, T * B_LOC)).astype(
                np.float16)
        s0c = s0[bc:bc + B_LOC]                             # [16, 512]
        s0T = np.ascontiguousarray(
            s0c.T.reshape(4, 128, B_LOC).transpose(1, 0, 2).reshape(128, 64)
        ).astype(np.float16)
        in_maps.append({
            "xT": xT, "pw": pw, "bias": bias, "aw": aw, "s0T": s0T,
            "iden": IDEN,
        })

    res = run_bass_kernel_spmd(nc, in_maps, list(range(N_CORES)), trace=_trace)
    if _trace:
        kernel._last_res = res

    out = np.empty((B_FULL, T + 1, S_DIM), np.float32)
    for c in range(N_CORES):
        bc = c * B_LOC
        stg = np.asarray(res.results[c]["stg"]).astype(np.float32)
        out[bc:bc + B_LOC, 0] = s0[bc:bc + B_LOC]
        out[bc:bc + B_LOC, 1:] = (
            stg.reshape(128, T, 4, B_LOC).transpose(3, 1, 2, 0)
            .reshape(B_LOC, T, S_DIM))
    return out


# revision 14
# speedup vs baseline: 1.6286x; 1.1822x over previous
"""Trainium2 Bass kernel for the BinaryMechanismSSM problem.

Full inputs in, full outputs out. Batch (128) sharded 8 ways (16/core).

Per core:
  Phase 1: projections bx0/bx1/gx = x @ {B0,B1,G}^T + bias as fp16 matmuls
           over 512-token tiles. bx planes are staged to DRAM pre-scaled by
           SCALE=512 (bias folded in); the gate plane is sigmoid(gx) fp16.
  Phase 2: T sequential steps. State held as st[p, 16j+b] = s[b, 128j+p]
           (fp16 [128, 64] tile). A0/A1 are fp8e4 (scaled by SCALE) with
           fp16 rhs; per step 32 A-matmuls + 4 identity-injection matmuls
           accumulate into 4 PSUM quarter tiles (one per state chunk j).
           MM issue order (iden, k=0 blocks, k=1 blocks, then per-j k=2/3
           blocks) lets the per-chunk tanh (scale=1/SCALE) + gate blend
           (DVE, fp16) pipeline underneath the matmuls of later chunks and
           of the next step, keeping the PE continuously busy (HAM-warm).
           States staged out fp16 every 4 steps; host re-layouts to
           [B, T+1, S] fp32.
"""
import numpy as np

B_FULL = 128
T_FULL = 1024
I_DIM = 256
S_DIM = 512
N_CORES = 8
B_LOC = B_FULL // N_CORES  # 16
SCALE = 512.0

_cache = {}


def _build(alpha: float, z: int, T: int):
    import ml_dtypes  # noqa: F401  (ensures fp8 numpy dtypes exist)
    import concourse.bass as bass  # noqa: F401
    from concourse import bacc
    import concourse.mybir as mybir
    from concourse.tile import TileContext

    dt = mybir.dt
    AF = mybir.ActivationFunctionType
    ALU = mybir.AluOpType

    TOK = T * B_LOC          # tokens per core
    NTT = TOK // 512         # phase-1 token tiles
    NG = T // 16             # phase-2 step groups
    NMAT = 3 if z != 0 else 2          # number of projection matrices
    NREC = 2 if z != 0 else 1          # number of recurrence matrices
    W2 = NREC * 16           # psum quarter width (m, b)

    nc = bacc.Bacc("TRN2", target_bir_lowering=False, debug=False,
                   num_devices=N_CORES)

    xT_d = nc.declare_dram_parameter("xT", [2, 128, TOK], dt.float16, isOutput=False)
    pw_d = nc.declare_dram_parameter("pw", [128, NMAT * 2 * 4 * 128], dt.float16, isOutput=False)
    bias_d = nc.declare_dram_parameter("bias", [128, 4 * NMAT], dt.float32, isOutput=False)
    aw_d = nc.declare_dram_parameter("aw", [128, NREC * 16 * 128], dt.float8e4, isOutput=False)
    s0_d = nc.declare_dram_parameter("s0T", [128, 64], dt.float16, isOutput=False)
    iden_d = nc.declare_dram_parameter("iden", [128, 128], dt.float8e4, isOutput=False)
    stg_d = nc.declare_dram_parameter("stg", [128, T, 64], dt.float16, isOutput=True)

    with TileContext(nc) as tc:
      with tc.tile_pool(name="dram", bufs=1, space="DRAM") as dpool:
        projb_p = [[dpool.tile([128, TOK], dt.float16, tag=f"projb{m}{j}",
                               name=f"projb{m}{j}")
                    for j in range(4)] for m in range(NREC)]
        projg_p = [dpool.tile([128, TOK], dt.float16, tag=f"projg{j}",
                              name=f"projg{j}") for j in range(4)]
        # ---------------- Phase 1: projections ----------------
        with (
            tc.tile_pool(name="p1w", bufs=1) as p1w,
            tc.tile_pool(name="p1x", bufs=3) as p1x,
            tc.tile_pool(name="p1o", bufs=6) as p1o,
            tc.tile_pool(name="p1ps", bufs=8, space="PSUM") as p1ps,
        ):
            pw = p1w.tile([128, NMAT * 2 * 4 * 128], dt.float16)
            nc.sync.dma_start(pw[:], pw_d[:])
            bias = p1w.tile([128, 4 * NMAT], dt.float32)
            nc.sync.dma_start(bias[:], bias_d[:])

            for tt in range(NTT):
                xt = p1x.tile([128, 2 * 512], dt.float16, tag="xt")
                for i in range(2):
                    nc.sync.dma_start(xt[:, i * 512:(i + 1) * 512],
                                      xT_d[i, :, tt * 512:(tt + 1) * 512])
                for mat in range(NMAT):
                    for j in range(4):
                        ps = p1ps.tile([128, 512], dt.float32, tag="pps")
                        for i in range(2):
                            blk = ((mat * 2 + i) * 4 + j) * 128
                            nc.tensor.matmul(
                                ps[:], pw[:, blk:blk + 128],
                                xt[:, i * 512:(i + 1) * 512],
                                start=(i == 0), stop=(i == 1))
                        bj = bias[:, mat * 4 + j:mat * 4 + j + 1]
                        ot = p1o.tile([128, 512], dt.float16, tag="po")
                        if mat == NMAT - 1:
                            nc.scalar.activation(ot[:], ps[:], AF.Sigmoid,
                                                 bias=bj, scale=1.0)
                            nc.sync.dma_start(
                                projg_p[j][:, tt * 512:(tt + 1) * 512], ot[:])
                        else:
                            # bias_d already holds SCALE*b for these mats
                            nc.scalar.activation(ot[:], ps[:], AF.Identity,
                                                 bias=bj, scale=SCALE)
                            nc.sync.dma_start(
                                projb_p[mat][j][:, tt * 512:(tt + 1) * 512],
                                ot[:])

        # ---------------- Phase 2: recurrence ----------------
        with (
            tc.tile_pool(name="p2w", bufs=1) as p2w,
            tc.tile_pool(name="p2in", bufs=1) as p2in,
            tc.tile_pool(name="p2st", bufs=1) as p2st,
            tc.tile_pool(name="p2c", bufs=1) as p2c,
            tc.tile_pool(name="p2ps", bufs=1, space="PSUM") as p2ps,
        ):
            aw = p2w.tile([128, NREC * 16 * 128], dt.float8e4)
            nc.sync.dma_start(aw[:], aw_d[:])
            iden = p2w.tile([128, 128], dt.float8e4)
            nc.sync.dma_start(iden[:], iden_d[:])
            s0t = p2w.tile([128, 64], dt.float16)
            nc.sync.dma_start(s0t[:], s0_d[:])

            a0 = float(1.0 - alpha) if z != 0 else 1.0
            a1 = float(alpha)
            WH = NREC * 32            # psum half width: (m, j2, b)
            ME = NREC + 1             # extended m-dim: mats + carry slice

            # explicit rings (pool buf rotation serializes one generation
            # too tight on the hot path; rings decouple by construction)
            pss_r = [[p2ps.tile([128, WH], dt.float32, tag=f"ps{h}{r}",
                                name=f"ps{h}{r}") for r in range(4)]
                     for h in range(2)]
            ft_r = [p2c.tile([128, 2 * WH], dt.float16, tag=f"ft{r}",
                             name=f"ft{r}") for r in range(2)]
            u_r = [[p2c.tile([128, ME * 32], dt.float16, tag=f"u{h}{r}",
                             name=f"u{h}{r}") for r in range(2)]
                   for h in range(2)]
            ob_r = [p2st.tile([128, 4 * 64], dt.float16, tag=f"ob{r}",
                              name=f"ob{r}") for r in range(2)]
            pjb_r_ = [p2in.tile([128, NREC * 4 * 256], dt.float16,
                                tag=f"pjb{r}", name=f"pjb{r}")
                      for r in range(2)]
            pjg_r_ = [p2in.tile([128, 1024], dt.float16, tag=f"pjg{r}",
                                name=f"pjg{r}") for r in range(2)]
            gco_r_ = [p2in.tile([128, NREC * 1024], dt.float16,
                                tag=f"gco{r}", name=f"gco{r}")
                      for r in range(2)]
            g1m_r_ = [p2in.tile([128, 1024], dt.float16, tag=f"g1m{r}",
                                name=f"g1m{r}") for r in range(2)]

            def stage_group(g):
                """DMA-in staged planes + build gate coeffs for group g."""
                r = g % 2
                pjb, pjg = pjb_r_[r], pjg_r_[r]
                gco, g1m = gco_r_[r], g1m_r_[r]
                for m in range(NREC):
                    for j in range(4):
                        nc.sync.dma_start(
                            pjb[:, (m * 4 + j) * 256:(m * 4 + j + 1) * 256],
                            projb_p[m][j][:, g * 256:(g + 1) * 256])
                for j in range(4):
                    nc.sync.dma_start(
                        pjg[:, j * 256:(j + 1) * 256],
                        projg_p[j][:, g * 256:(g + 1) * 256])
                nc.vector.tensor_scalar_mul(gco[:, 0:1024], pjg[:], a0)
                if NREC == 2:
                    nc.vector.tensor_scalar_mul(gco[:, 1024:2048], pjg[:],
                                                a1)
                nc.vector.tensor_scalar(g1m[:], pjg[:], -1.0, 1.0,
                                        ALU.mult, ALU.add)

            stage_group(0)
            # init the t=-1 u-tiles: mat slices zero, carry slice = s0
            g1m0r = g1m_r_[0][:].rearrange("p (j t b) -> p j t b", j=4, t=16)
            for h in range(2):
                ui = u_r[h][1]
                nc.vector.memset(ui[:, 0:NREC * 32], 0.0)
                nc.vector.tensor_copy(ui[:, NREC * 32:ME * 32],
                                      s0t[:, h * 32:(h + 1) * 32])
            st_prev = s0t
            for g in range(NG):
                r = g % 2
                pjbr = pjb_r_[r][:].rearrange("p (m j t b) -> p m j t b",
                                              m=NREC, j=4, t=16, b=16)
                gcor = gco_r_[r][:].rearrange("p (m j t b) -> p m j t b",
                                              m=NREC, j=4, t=16, b=16)
                g1mr = g1m_r_[r][:].rearrange("p (j t b) -> p j t b",
                                              j=4, t=16)

                for tt in range(16):
                    t = g * 16 + tt
                    pss = [pss_r[h][t % 4] for h in range(2)]
                    ft = ft_r[t % 2]
                    uu = [u_r[h][t % 2] for h in range(2)]
                    up = [u_r[h][(t + 1) % 2] for h in range(2)]
                    upr = [up[h][:].rearrange("p (m j b) -> p m j b",
                                              m=ME, j=2) for h in range(2)]
                    obuf = ob_r[(t // 4) % 2]
                    st_new = obuf[:, (tt % 4) * 64:(tt % 4) * 64 + 64]

                    # carry slice of this step's u: (1-g) * s_{t-1}
                    for h in range(2):
                        nc.vector.tensor_tensor(
                            uu[h][:, NREC * 32:ME * 32]
                            .rearrange("p (j b) -> p j b", j=2),
                            st_prev[:, h * 32:(h + 1) * 32]
                            .rearrange("p (j b) -> p j b", j=2),
                            g1mr[:, 2 * h:2 * h + 2, tt, :], ALU.mult)

                    # inject bx via fp8-identity matmuls (rhs fp16, exact)
                    for h in range(2):
                        nc.tensor.matmul(
                            pss[h][:].rearrange("p (m j b) -> p m j b",
                                                m=NREC, j=2),
                            iden[:], pjbr[:, :, 2 * h:2 * h + 2, tt, :],
                            start=True, stop=False)

                    # A-matmuls: rhs = 3 m-slices of the previous step's
                    # u-tile (u0, u1, carry); a stride-0 broadcast out AP
                    # makes the PE accumulate them = A @ s_{t-1}
                    def a_mm(h, k, last=False):
                        hk, jk = k // 2, k % 2
                        rhs = upr[hk][:, :, jk, :]
                        for m in range(NREC):
                            for jj in range(2):
                                j = 2 * h + jj
                                blk = (m * 16 + k * 4 + j) * 128
                                out = pss[h][:, m * 32 + jj * 16:
                                             m * 32 + jj * 16 + 16] \
                                    .unsqueeze(1).broadcast_to([128, ME, 16])
                                nc.tensor.matmul(
                                    out, aw[:, blk:blk + 128], rhs,
                                    start=False,
                                    stop=(last and m == NREC - 1
                                          and jj == 1))

                    a_mm(0, 0)
                    a_mm(0, 1)
                    a_mm(0, 2)
                    a_mm(0, 3, last=True)
                    nc.scalar.activation(ft[:, 0:WH], pss[0][:], AF.Tanh,
                                         scale=1.0 / SCALE)
                    a_mm(1, 0)
                    a_mm(1, 1)
                    a_mm(1, 2)
                    a_mm(1, 3, last=True)
                    nc.scalar.activation(ft[:, WH:2 * WH], pss[1][:],
                                         AF.Tanh, scale=1.0 / SCALE)

                    # group staging prefetch mid-step-loop (off the path)
                    if tt == 8 and g + 1 < NG:
                        stage_group(g + 1)

                    # DVE on-path: u_m = ft * gco for both halves
                    for h in range(2):
                        nc.vector.tensor_tensor(
                            uu[h][:, 0:NREC * 32]
                            .rearrange("p (m j b) -> p m j b", m=NREC, j=2),
                            ft[:, h * WH:(h + 1) * WH]
                            .rearrange("p (m j b) -> p m j b", m=NREC, j=2),
                            gcor[:, :, 2 * h:2 * h + 2, tt, :], ALU.mult)
                    # off-path: materialize s_t for output + next carry
                    for h in range(2):
                        with nc.allow_low_precision("fp16 3-term gate sum"):
                            nc.vector.tensor_reduce(
                                st_new[:, h * 32:(h + 1) * 32]
                                .rearrange("p (j b) -> p j b", j=2),
                                uu[h][:].rearrange("p (m j b) -> p j b m",
                                                   m=ME, j=2),
                                mybir.AxisListType.X, ALU.add)

                    st_prev = st_new
                    if tt % 4 == 3:
                        nc.sync.dma_start(
                            stg_d[:, t - 3:t + 1, :],
                            obuf[:].rearrange("p (t c) -> p t c", t=4))

    nc.compile()
    return nc


def _pack_lhsT_blocks(W, kdim, mdim, dtype):
    """W: [mdim*128, kdim*128]; returns [128, kdim*mdim*128] with block
    (k, j) at cols (k*mdim+j)*128 equal to W[j-chunk, k-chunk].T."""
    nk, nj = kdim, mdim
    out = np.zeros((128, nk * nj * 128), dtype=dtype)
    for k in range(nk):
        for j in range(nj):
            blk = W[j * 128:(j + 1) * 128, k * 128:(k + 1) * 128].T
            out[:, (k * nj + j) * 128:(k * nj + j + 1) * 128] = blk
    return np.ascontiguousarray(out)


def kernel(x_seq, s0, A0_w, B0_w, B0_b, A1_w, B1_w, B1_b, gate_w, gate_b,
           alpha, z, _T=None, _trace=False):
    import ml_dtypes
    from concourse.bass_utils import run_bass_kernel_spmd

    T = int(_T or T_FULL)
    alpha_f = float(np.asarray(alpha))
    z_i = int(np.asarray(z))

    key = (alpha_f, z_i, T)
    if key not in _cache:
        _cache[key] = _build(alpha_f, z_i, T)
    nc = _cache[key]

    NMAT = 3 if z_i != 0 else 2
    NREC = 2 if z_i != 0 else 1

    x_seq = np.asarray(x_seq, dtype=np.float32)
    s0 = np.asarray(s0, dtype=np.float32)

    # ---- shared (replicated) weight packing ----
    mats = [np.asarray(B0_w), np.asarray(B1_w), np.asarray(gate_w)][:NMAT] \
        if z_i != 0 else [np.asarray(B0_w), np.asarray(gate_w)]
    biases = [np.asarray(B0_b), np.asarray(B1_b), np.asarray(gate_b)][:NMAT] \
        if z_i != 0 else [np.asarray(B0_b), np.asarray(gate_b)]
    pw = np.concatenate(
        [_pack_lhsT_blocks(W.astype(np.float32), 2, 4, np.float32)
         for W in mats], axis=1).astype(np.float16)
    pw = np.ascontiguousarray(pw)

    # bias for the bx mats is pre-scaled by SCALE (folded into phase-1 ACT)
    bias = np.zeros((128, 4 * NMAT), np.float32)
    for mi, bvec in enumerate(biases):
        scl = 1.0 if mi == NMAT - 1 else SCALE
        bias[:, mi * 4:(mi + 1) * 4] = \
            (scl * bvec.astype(np.float32)).reshape(4, 128).T

    recs = [np.asarray(A0_w)] if z_i == 0 else [np.asarray(A0_w), np.asarray(A1_w)]
    aw = np.concatenate(
        [_pack_lhsT_blocks(A.astype(np.float32), 4, 4, np.float32)
         for A in recs], axis=1) * SCALE
    aw = np.ascontiguousarray(np.clip(aw, -240.0, 240.0)).astype(
        ml_dtypes.float8_e4m3)

    IDEN = np.ascontiguousarray(np.eye(128).astype(ml_dtypes.float8_e4m3))

    # ---- per-core inputs ----
    in_maps = []
    for c in range(N_CORES):
        bc = c * B_LOC
        xc = x_seq[bc:bc + B_LOC, :T]                       # [16, T, 256]
        xT = np.ascontiguousarray(
            xc.transpose(2, 1, 0).reshape(2, 128, T * B_LOC)).astype(
                np.float16)
        s0c = s0[bc:bc + B_LOC]                             # [16, 512]
        s0T = np.ascontiguousarray(
            s0c.T.reshape(4, 128, B_LOC).transpose(1, 0, 2).reshape(128, 64)
        ).astype(np.float16)
        in_maps.append({
            "xT": xT, "pw": pw, "bias": bias, "aw": aw, "s0T": s0T,
            "iden": IDEN,
        })

    res = run_bass_kernel_spmd(nc, in_maps, list(range(N_CORES)), trace=_trace)
    if _trace:
        kernel._last_res = res

    out = np.empty((B_FULL, T + 1, S_DIM), np.float32)
    for c in range(N_CORES):
        bc = c * B_LOC
        stg = np.asarray(res.results[c]["stg"]).astype(np.float32)
        out[bc:bc + B_LOC, 0] = s0[bc:bc + B_LOC]
        out[bc:bc + B_LOC, 1:] = (
            stg.reshape(128, T, 4, B_LOC).transpose(3, 1, 2, 0)
            .reshape(B_LOC, T, S_DIM))
    return out


# revision 18
# speedup vs baseline: 1.9255x; 1.1823x over previous
"""Trainium2 Bass kernel for the BinaryMechanismSSM problem.

Full inputs in, full outputs out. Batch (128) sharded 8 ways (16/core).

Per core:
  Phase 1: projections bx0/bx1/gx = x @ {B0,B1,G}^T + bias as fp16 matmuls
           over 512-token tiles. bx planes are staged to DRAM pre-scaled by
           SCALE=512 (bias folded in); the gate plane is sigmoid(gx) fp16.
  Phase 2: T sequential steps. State held as st[p, 16j+b] = s[b, 128j+p]
           (fp16 [128, 64] tile). A0/A1 are fp8e4 (scaled by SCALE) with
           fp16 rhs; per step 32 A-matmuls + 4 identity-injection matmuls
           accumulate into 4 PSUM quarter tiles (one per state chunk j).
           MM issue order (iden, k=0 blocks, k=1 blocks, then per-j k=2/3
           blocks) lets the per-chunk tanh (scale=1/SCALE) + gate blend
           (DVE, fp16) pipeline underneath the matmuls of later chunks and
           of the next step, keeping the PE continuously busy (HAM-warm).
           States staged out fp16 every 4 steps; host re-layouts to
           [B, T+1, S] fp32.
"""
import numpy as np

B_FULL = 128
T_FULL = 1024
I_DIM = 256
S_DIM = 512
N_CORES = 8
B_LOC = B_FULL // N_CORES  # 16
SCALE = 512.0

_cache = {}


def _build(alpha: float, z: int, T: int):
    import ml_dtypes  # noqa: F401  (ensures fp8 numpy dtypes exist)
    import concourse.bass as bass  # noqa: F401
    from concourse import bacc
    import concourse.mybir as mybir
    from concourse.tile import TileContext

    dt = mybir.dt
    AF = mybir.ActivationFunctionType
    ALU = mybir.AluOpType

    TOK = T * B_LOC          # tokens per core
    NTT = TOK // 512         # phase-1 token tiles
    NG = T // 16             # phase-2 step groups
    NMAT = 3 if z != 0 else 2          # number of projection matrices
    NREC = 2 if z != 0 else 1          # number of recurrence matrices
    W2 = NREC * 16           # psum quarter width (m, b)

    nc = bacc.Bacc("TRN2", target_bir_lowering=False, debug=False,
                   num_devices=N_CORES)

    xT_d = nc.declare_dram_parameter("xT", [2, 128, TOK], dt.float16, isOutput=False)
    pw_d = nc.declare_dram_parameter("pw", [128, NMAT * 2 * 4 * 128], dt.float16, isOutput=False)
    bias_d = nc.declare_dram_parameter("bias", [128, 4 * NMAT], dt.float32, isOutput=False)
    aw_d = nc.declare_dram_parameter("aw", [128, NREC * 16 * 128], dt.float8e4, isOutput=False)
    s0_d = nc.declare_dram_parameter("s0T", [128, 64], dt.float16, isOutput=False)
    iden_d = nc.declare_dram_parameter("iden", [128, 128], dt.float8e4, isOutput=False)
    stg_d = nc.declare_dram_parameter("stg", [128, T, 64], dt.float16, isOutput=True)

    with TileContext(nc) as tc:
        # ---------------- fused recurrence + JIT projections ----------
        with (
            tc.tile_pool(name="p2w", bufs=1) as p2w,
            tc.tile_pool(name="p2in", bufs=1) as p2in,
            tc.tile_pool(name="p2st", bufs=1) as p2st,
            tc.tile_pool(name="p2c", bufs=1) as p2c,
            tc.tile_pool(name="p2ps", bufs=1, space="PSUM") as p2ps,
        ):
            aw = p2w.tile([128, NREC * 16 * 128], dt.float8e4)
            nc.sync.dma_start(aw[:], aw_d[:])
            iden = p2w.tile([128, 128], dt.float8e4)
            nc.sync.dma_start(iden[:], iden_d[:])
            s0t = p2w.tile([128, 64], dt.float16)
            nc.sync.dma_start(s0t[:], s0_d[:])
            pw = p2w.tile([128, NMAT * 2 * 4 * 128], dt.float16)
            nc.sync.dma_start(pw[:], pw_d[:])
            bias = p2w.tile([128, 4 * NMAT], dt.float32)
            nc.sync.dma_start(bias[:], bias_d[:])

            a0 = float(1.0 - alpha) if z != 0 else 1.0
            a1 = float(alpha)
            WH = NREC * 32            # psum half width: (m, j2, b)
            ME = NREC + 1             # extended m-dim: mats + carry slice

            # explicit rings (pool buf rotation serializes one generation
            # too tight on the hot path; rings decouple by construction)
            pss_r = [[p2ps.tile([128, WH], dt.float32, tag=f"ps{h}{r}",
                                name=f"ps{h}{r}") for r in range(3)]
                     for h in range(2)]
            pps_r = [p2ps.tile([128, 256], dt.float32, tag=f"pp{r}",
                               name=f"pp{r}") for r in range(2)]
            ft_r = [p2c.tile([128, 2 * WH], dt.float16, tag=f"ft{r}",
                             name=f"ft{r}") for r in range(2)]
            u_r = [[p2c.tile([128, ME * 32], dt.float16, tag=f"u{h}{r}",
                             name=f"u{h}{r}") for r in range(2)]
                   for h in range(2)]
            ob_r = [p2st.tile([128, 4 * 64], dt.float16, tag=f"ob{r}",
                              name=f"ob{r}") for r in range(2)]
            xg_r = [p2in.tile([128, 512], dt.float16, tag=f"xg{r}",
                              name=f"xg{r}") for r in range(2)]
            pjb_r_ = [p2in.tile([128, NREC * 4 * 256], dt.float16,
                                tag=f"pjb{r}", name=f"pjb{r}")
                      for r in range(2)]
            pjg_r_ = [p2in.tile([128, 1024], dt.float16, tag=f"pjg{r}",
                                name=f"pjg{r}") for r in range(2)]
            gco_r_ = [p2in.tile([128, NREC * 1024], dt.float16,
                                tag=f"gco{r}", name=f"gco{r}")
                      for r in range(2)]
            g1m_r_ = [p2in.tile([128, 1024], dt.float16, tag=f"g1m{r}",
                                name=f"g1m{r}") for r in range(2)]

            def stage_xdma(g):
                """Prefetch this group's 256 tokens of x into SBUF."""
                r = g % 2
                for i in range(2):
                    nc.sync.dma_start(xg_r[r][:, i * 256:(i + 1) * 256],
                                      xT_d[i, :, g * 256:(g + 1) * 256])

            # proj chunk order: gate first so gco/g1m can build early
            chunks = [(NMAT - 1, j) for j in range(4)] + \
                     [(m, j) for m in range(NREC) for j in range(4)]

            def stage_proj(g, c):
                """JIT projection chunk c (one (mat, j) pair) for group g."""
                r = g % 2
                mat, j = chunks[c]
                ps = pps_r[c % 2]
                for i in range(2):
                    blk = ((mat * 2 + i) * 4 + j) * 128
                    nc.tensor.matmul(ps[:], pw[:, blk:blk + 128],
                                     xg_r[r][:, i * 256:(i + 1) * 256],
                                     start=(i == 0), stop=(i == 1))
                bj = bias[:, mat * 4 + j:mat * 4 + j + 1]
                if mat == NMAT - 1:
                    nc.scalar.activation(
                        pjg_r_[r][:, j * 256:(j + 1) * 256], ps[:],
                        AF.Sigmoid, bias=bj, scale=1.0)
                else:
                    nc.scalar.activation(
                        pjb_r_[r][:, (mat * 4 + j) * 256:
                                 (mat * 4 + j + 1) * 256], ps[:],
                        AF.Identity, bias=bj, scale=SCALE)

            def stage_gco(g, which):
                """Gate coefficient planes for group g (after its pjg)."""
                r = g % 2
                if which == 0:
                    nc.vector.tensor_scalar_mul(gco_r_[r][:, 0:1024],
                                                pjg_r_[r][:], a0)
                    if NREC == 2:
                        nc.vector.tensor_scalar_mul(
                            gco_r_[r][:, 1024:2048], pjg_r_[r][:], a1)
                else:
                    nc.vector.tensor_scalar(g1m_r_[r][:], pjg_r_[r][:],
                                            -1.0, 1.0, ALU.mult, ALU.add)

            # prologue: fully stage group 0
            stage_xdma(0)
            for c in range(4 + NREC * 4):
                stage_proj(0, c)
            stage_gco(0, 0)
            stage_gco(0, 1)
            # init the t=-1 u-tiles: mat slices zero, carry slice = s0
            g1m0r = g1m_r_[0][:].rearrange("p (j t b) -> p j t b", j=4, t=16)
            for h in range(2):
                ui = u_r[h][1]
                nc.vector.memset(ui[:, 0:NREC * 32], 0.0)
                nc.vector.tensor_copy(ui[:, NREC * 32:ME * 32],
                                      s0t[:, h * 32:(h + 1) * 32])
            st_prev = s0t
            for g in range(NG):
                r = g % 2
                pjbr = pjb_r_[r][:].rearrange("p (m j t b) -> p m j t b",
                                              m=NREC, j=4, t=16, b=16)
                gcor = gco_r_[r][:].rearrange("p (m j t b) -> p m j t b",
                                              m=NREC, j=4, t=16, b=16)
                g1mr = g1m_r_[r][:].rearrange("p (j t b) -> p j t b",
                                              j=4, t=16)

                for tt in range(16):
                    t = g * 16 + tt
                    pss = [pss_r[h][t % 3] for h in range(2)]
                    ft = ft_r[t % 2]
                    uu = [u_r[h][t % 2] for h in range(2)]
                    up = [u_r[h][(t + 1) % 2] for h in range(2)]
                    upr = [up[h][:].rearrange("p (m j b) -> p m j b",
                                              m=ME, j=2) for h in range(2)]
                    obuf = ob_r[(t // 4) % 2]
                    st_new = obuf[:, (tt % 4) * 64:(tt % 4) * 64 + 64]

                    # carry slice of this step's u: (1-g) * s_{t-1}
                    for h in range(2):
                        nc.vector.tensor_tensor(
                            uu[h][:, NREC * 32:ME * 32]
                            .rearrange("p (j b) -> p j b", j=2),
                            st_prev[:, h * 32:(h + 1) * 32]
                            .rearrange("p (j b) -> p j b", j=2),
                            g1mr[:, 2 * h:2 * h + 2, tt, :], ALU.mult)

                    # inject bx via fp8-identity matmuls (rhs fp16, exact)
                    for h in range(2):
                        nc.tensor.matmul(
                            pss[h][:].rearrange("p (m j b) -> p m j b",
                                                m=NREC, j=2),
                            iden[:], pjbr[:, :, 2 * h:2 * h + 2, tt, :],
                            start=True, stop=False)

                    # A-matmuls: rhs = 3 m-slices of the previous step's
                    # u-tile (u0, u1, carry); a stride-0 broadcast out AP
                    # makes the PE accumulate them = A @ s_{t-1}
                    def a_mm(h, k, last=False):
                        hk, jk = k // 2, k % 2
                        rhs = upr[hk][:, :, jk, :]
                        for m in range(NREC):
                            for jj in range(2):
                                j = 2 * h + jj
                                blk = (m * 16 + k * 4 + j) * 128
                                out = pss[h][:, m * 32 + jj * 16:
                                             m * 32 + jj * 16 + 16] \
                                    .unsqueeze(1).broadcast_to([128, ME, 16])
                                nc.tensor.matmul(
                                    out, aw[:, blk:blk + 128], rhs,
                                    start=False,
                                    stop=(last and m == NREC - 1
                                          and jj == 1))

                    a_mm(0, 0)
                    a_mm(0, 1)
                    a_mm(0, 2)
                    a_mm(0, 3, last=True)
                    nc.scalar.activation(ft[:, 0:WH], pss[0][:], AF.Tanh,
                                         scale=1.0 / SCALE)
                    a_mm(1, 0)
                    a_mm(1, 1)
                    a_mm(1, 2)
                    a_mm(1, 3, last=True)
                    nc.scalar.activation(ft[:, WH:2 * WH], pss[1][:],
                                         AF.Tanh, scale=1.0 / SCALE)

                    # JIT projections for the next group, spread across
                    # this group's steps (fills PE/ACT idle, keeps PE warm)
                    if g + 1 < NG:
                        if tt == 0:
                            stage_xdma(g + 1)
                        elif 2 <= tt < 2 + 4 + NREC * 4:
                            stage_proj(g + 1, tt - 2)
                        elif tt == 14:
                            stage_gco(g + 1, 0)
                        elif tt == 15:
                            stage_gco(g + 1, 1)

                    # DVE on-path: u_m = ft * gco for both halves
                    for h in range(2):
                        nc.vector.tensor_tensor(
                            uu[h][:, 0:NREC * 32]
                            .rearrange("p (m j b) -> p m j b", m=NREC, j=2),
                            ft[:, h * WH:(h + 1) * WH]
                            .rearrange("p (m j b) -> p m j b", m=NREC, j=2),
                            gcor[:, :, 2 * h:2 * h + 2, tt, :], ALU.mult)
                    # off-path: materialize s_t for output + next carry
                    for h in range(2):
                        with nc.allow_low_precision("fp16 3-term gate sum"):
                            nc.vector.tensor_reduce(
                                st_new[:, h * 32:(h + 1) * 32]
                                .rearrange("p (j b) -> p j b", j=2),
                                uu[h][:].rearrange("p (m j b) -> p j b m",
                                                   m=ME, j=2),
                                mybir.AxisListType.X, ALU.add)

                    st_prev = st_new
                    if tt % 4 == 3:
                        nc.sync.dma_start(
                            stg_d[:, t - 3:t + 1, :],
                            obuf[:].rearrange("p (t c) -> p t c", t=4))

    nc.compile()
    return nc


def _pack_lhsT_blocks(W, kdim, mdim, dtype):
    """W: [mdim*128, kdim*128]; returns [128, kdim*mdim*128] with block
    (k, j) at cols (k*mdim+j)*128 equal to W[j-chunk, k-chunk].T."""
    nk, nj = kdim, mdim
    out = np.zeros((128, nk * nj * 128), dtype=dtype)
    for k in range(nk):
        for j in range(nj):
            blk = W[j * 128:(j + 1) * 128, k * 128:(k + 1) * 128].T
            out[:, (k * nj + j) * 128:(k * nj + j + 1) * 128] = blk
    return np.ascontiguousarray(out)


def kernel(x_seq, s0, A0_w, B0_w, B0_b, A1_w, B1_w, B1_b, gate_w, gate_b,
           alpha, z, _T=None, _trace=False):
    import ml_dtypes
    from concourse.bass_utils import run_bass_kernel_spmd

    T = int(_T or T_FULL)
    alpha_f = float(np.asarray(alpha))
    z_i = int(np.asarray(z))

    key = (alpha_f, z_i, T)
    if key not in _cache:
        _cache[key] = _build(alpha_f, z_i, T)
    nc = _cache[key]

    NMAT = 3 if z_i != 0 else 2
    NREC = 2 if z_i != 0 else 1

    x_seq = np.asarray(x_seq, dtype=np.float32)
    s0 = np.asarray(s0, dtype=np.float32)

    # ---- shared (replicated) weight packing ----
    mats = [np.asarray(B0_w), np.asarray(B1_w), np.asarray(gate_w)][:NMAT] \
        if z_i != 0 else [np.asarray(B0_w), np.asarray(gate_w)]
    biases = [np.asarray(B0_b), np.asarray(B1_b), np.asarray(gate_b)][:NMAT] \
        if z_i != 0 else [np.asarray(B0_b), np.asarray(gate_b)]
    pw = np.concatenate(
        [_pack_lhsT_blocks(W.astype(np.float32), 2, 4, np.float32)
         for W in mats], axis=1).astype(np.float16)
    pw = np.ascontiguousarray(pw)

    # bias for the bx mats is pre-scaled by SCALE (folded into phase-1 ACT)
    bias = np.zeros((128, 4 * NMAT), np.float32)
    for mi, bvec in enumerate(biases):
        scl = 1.0 if mi == NMAT - 1 else SCALE
        bias[:, mi * 4:(mi + 1) * 4] = \
            (scl * bvec.astype(np.float32)).reshape(4, 128).T

    recs = [np.asarray(A0_w)] if z_i == 0 else [np.asarray(A0_w), np.asarray(A1_w)]
    aw = np.concatenate(
        [_pack_lhsT_blocks(A.astype(np.float32), 4, 4, np.float32)
         for A in recs], axis=1) * SCALE
    aw = np.ascontiguousarray(np.clip(aw, -240.0, 240.0)).astype(
        ml_dtypes.float8_e4m3)

    IDEN = np.ascontiguousarray(np.eye(128).astype(ml_dtypes.float8_e4m3))

    # ---- per-core inputs ----
    in_maps = []
    for c in range(N_CORES):
        bc = c * B_LOC
        xc = x_seq[bc:bc + B_LOC, :T]                       # [16, T, 256]
        xT = np.ascontiguousarray(
            xc.transpose(2, 1, 0).reshape(2, 128, T * B_LOC)).astype(
                np.float16)
        s0c = s0[bc:bc + B_LOC]                             # [16, 512]
        s0T = np.ascontiguousarray(
            s0c.T.reshape(4, 128, B_LOC).transpose(1, 0, 2).reshape(128, 64)
        ).astype(np.float16)
        in_maps.append({
            "xT": xT, "pw": pw, "bias": bias, "aw": aw, "s0T": s0T,
            "iden": IDEN,
        })

    res = run_bass_kernel_spmd(nc, in_maps, list(range(N_CORES)), trace=_trace)
    if _trace:
        kernel._last_res = res

    out = np.empty((B_FULL, T + 1, S_DIM), np.float32)
    for c in range(N_CORES):
        bc = c * B_LOC
        stg = np.asarray(res.results[c]["stg"]).astype(np.float32)
        out[bc:bc + B_LOC, 0] = s0[bc:bc + B_LOC]
        out[bc:bc + B_LOC, 1:] = (
            stg.reshape(128, T, 4, B_LOC).transpose(3, 1, 2, 0)
            .reshape(B_LOC, T, S_DIM))
    return out
